# revision 1
# baseline (speedup 1.0000x reference)
"""Two-layer GCN (PyG GCNConv-style) on 8 Trainium2 NeuronCores.

Strategy (per the sharding hint): nodes are partitioned across the 8
cores (load-balanced into 128-row tiles by in-degree), edges are
partitioned by destination node so the segment-sum is local to the
destination's core.  Each GCN layer is: local GEMM (transform), an
AllGather of the transformed features, then a local gather+weighted
segment-sum over the incoming edges.

The segment-sum runs on the TensorEngine: for each destination tile of
128 nodes, its incoming edges (chunked by 128) are gathered with bulk
dma_gather into SBUF [128edges x F] per chunk, and contracted with a
host-built indicator matrix S [128edges x 128dst] (value = the symmetric
GCN norm for that edge) accumulating into PSUM [128dst x F].

dma_gather takes int16 row indices, so the gathered table is addressed
through two overlapping <=32767-row windows (A = [0, WCAP),
B = [NG-WCAP, NG)); each destination tile's edges are split between the
windows (the overlap zone gives freedom to balance the split so no extra
padding chunks are needed).  Self-loop edges are not gathered at all:
a destination tile's own rows are contiguous in the local h, so they are
fetched with one plain DMA and folded in as an extra (diagonal) chunk.

Matmul inputs are typed float32r (TF32): full fp32 data, 4x matmul rate
at free-dim >= 256, ~1e-3 rounding in the multiplies only.
"""

import numpy as np

P = 128
N_CORES = 8
WINDOW_CAP = 32512  # dma_gather int16 window (multiple of 128, <= 32767)
USE_F32R = True

_prog_cache = {}


# ---------------------------------------------------------------- host side


def _preprocess(x, edge_index):
    """Partition nodes/edges, build per-core device arrays."""
    x = np.ascontiguousarray(np.asarray(x, dtype=np.float32))
    ei = np.asarray(edge_index)
    N, IN = x.shape

    src = ei[0].astype(np.int64)
    dst = ei[1].astype(np.int64)

    deg = 1 + np.bincount(dst, minlength=N)  # with self loop, >= 1
    dinv = (1.0 / np.sqrt(deg.astype(np.float64))).astype(np.float32)
    norm = dinv[src] * dinv[dst]  # non-self edges only
    norm_self = (dinv * dinv).astype(np.float32)

    npc_nodes = -(-N // N_CORES)
    T = -(-npc_nodes // P)  # dst tiles per core
    NPC = T * P  # node slots per core
    n_tiles = N_CORES * T
    NG = n_tiles * P  # global node slots

    # --- pack nodes into tiles, balancing per-tile in-degree (LPT) ----
    import heapq

    degg = deg - 1  # gathered (non-self) in-degree
    tile_of = np.empty(N, dtype=np.int64)
    pos_of = np.empty(N, dtype=np.int64)
    counts = np.zeros(n_tiles, dtype=np.int64)
    loads = np.zeros(n_tiles, dtype=np.int64)
    order = np.argsort(-degg, kind="stable")
    heap = [(0, t) for t in range(n_tiles)]
    heapq.heapify(heap)
    deg_l = degg[order]
    for i in range(N):
        v = order[i]
        while True:
            load, t = heapq.heappop(heap)
            if counts[t] < P:
                break
        tile_of[v] = t
        pos_of[v] = counts[t]
        counts[t] += 1
        load += int(deg_l[i])
        loads[t] = load
        if counts[t] < P:
            heapq.heappush(heap, (load, t))

    # repair pass: move small nodes off overloaded tiles to reach the
    # ideal chunk count ceil(total/(n_tiles*P)) if possible
    K_ideal = max(1, int(-(-int(degg.sum()) // (n_tiles * P))))
    target = K_ideal * P
    if loads.max() > target:
        by_tile = [[] for _ in range(n_tiles)]
        for i in range(N - 1, -1, -1):  # ascending degree order
            by_tile[tile_of[order[i]]].append(order[i])
        free = [(loads[t], t) for t in range(n_tiles)
                if counts[t] < P and loads[t] < target]
        heapq.heapify(free)
        for t_over in np.flatnonzero(loads > target):
            stack = by_tile[t_over]
            si = 0
            while loads[t_over] > target and si < len(stack) and free:
                v = stack[si]
                si += 1
                d = int(degg[v])
                moved = False
                tried = []
                while free:
                    lo, t2 = heapq.heappop(free)
                    if lo != loads[t2] or counts[t2] >= P:
                        continue  # stale
                    if loads[t2] + d <= target:
                        tile_of[v] = t2
                        pos_of[v] = counts[t2]
                        counts[t2] += 1
                        loads[t2] += d
                        loads[t_over] -= d
                        moved = True
                        if counts[t2] < P and loads[t2] < target:
                            heapq.heappush(free, (loads[t2], t2))
                        break
                    tried.append((lo, t2))
                for it in tried:
                    heapq.heappush(free, it)
                if not moved:
                    break
        # compact positions of overloaded tiles (pos_of may have holes now)
        for t in range(n_tiles):
            pass
        # recompute pos_of consistently
        ordv = np.lexsort((np.arange(N), tile_of))
        pos = np.empty(N, dtype=np.int64)
        tt = tile_of[ordv]
        st = np.zeros(n_tiles + 1, dtype=np.int64)
        np.cumsum(np.bincount(tt, minlength=n_tiles), out=st[1:])
        pos[ordv] = np.arange(N) - st[tt]
        pos_of = pos

    K = max(1, int(-(-loads.max() // P)))  # min gather chunks per dst tile

    row_of = tile_of * P + pos_of  # global new row of each node

    # --- per-edge placement (non-self edges) --------------------------
    e_tile = tile_of[dst]
    e_dslot = pos_of[dst].astype(np.int64)
    e_srcrow = row_of[src]

    sort_idx = np.lexsort((e_srcrow, e_tile))
    e_tile = e_tile[sort_idx]
    e_dslot = e_dslot[sort_idx]
    e_srcrow = e_srcrow[sort_idx]
    e_norm = norm[sort_idx]
    nE = len(e_tile)

    # --- window split (dma_gather int16 limit) ------------------------
    WA = min(WINDOW_CAP, NG)  # window A = rows [0, WA)
    WB_off = max(NG - WINDOW_CAP, 0)  # window B = rows [WB_off, NG)
    use_B = WB_off > 0

    tile_n = np.bincount(e_tile, minlength=n_tiles)
    if use_B:
        mustA = e_srcrow < WB_off
        mustB = e_srcrow >= WA
        flex = ~mustA & ~mustB
        cntA = np.bincount(e_tile[mustA], minlength=n_tiles)
        cntB = np.bincount(e_tile[mustB], minlength=n_tiles)
        # find (K_A, K_B) with K_A+K_B minimal and all tiles feasible
        found = None
        K_tot = K
        while found is None:
            mid = -(-K_tot // 2)
            for d in range(K_tot + 1):
                for K_A in {mid + d, mid - d}:
                    if not 0 <= K_A <= K_tot:
                        continue
                    K_B = K_tot - K_A
                    if (
                        cntA.max() <= K_A * P
                        and cntB.max() <= K_B * P
                        and tile_n.max() <= (K_A + K_B) * P
                    ):
                        found = (K_A, K_B)
                        break
                if found:
                    break
            if not found:
                K_tot += 1
        K_A, K_B = found
        capB = K_B * P
        # how many of each tile's flex edges go to window A
        nA_t = np.minimum(K_A * P, cntA + np.bincount(
            e_tile[flex], minlength=n_tiles))
        nA_t = np.maximum(nA_t, tile_n - capB)
        flexA_quota = nA_t - cntA
        # rank of each flex edge within its tile (sorted order preserved)
        flex_idx = np.flatnonzero(flex)
        ft = e_tile[flex_idx]
        fstart = np.zeros(n_tiles + 1, dtype=np.int64)
        np.cumsum(np.bincount(ft, minlength=n_tiles), out=fstart[1:])
        frank = np.arange(len(ft)) - fstart[ft]
        toA = mustA.copy()
        toA[flex_idx[frank < flexA_quota[ft]]] = True
    else:
        K_A, K_B = K, 0
        toA = np.ones(nE, dtype=bool)
    K_tot = K_A + K_B
    KC = K_tot + 1  # chunk columns per tile incl. the self chunk

    # --- chunk/slot assignment within each (tile, window) -------------
    e_j = np.empty(nE, dtype=np.int64)  # position within its window list
    e_val = np.empty(nE, dtype=np.int64)  # int16 index value
    for is_A in (True, False):
        m = toA if is_A else ~toA
        if not m.any():
            continue
        idxs = np.flatnonzero(m)
        t_sel = e_tile[idxs]
        start = np.zeros(n_tiles + 1, dtype=np.int64)
        np.cumsum(np.bincount(t_sel, minlength=n_tiles), out=start[1:])
        e_j[idxs] = np.arange(len(idxs)) - start[t_sel]
        e_val[idxs] = e_srcrow[idxs] - (0 if is_A else WB_off)

    e_kloc = e_j // P  # chunk within window
    e_p = e_j % P
    e_chunk = np.where(toA, e_kloc, K_A + e_kloc)  # chunk within tile

    e_core = e_tile // T
    e_t_in_core = e_tile % T
    e_col = e_t_in_core * KC + e_chunk  # chunk column within core

    # idx table: per gather block of 8*K_w columns; value j at [j%16, j//16],
    # replicated across the 8 groups of 16 partitions (one per Q7 core)
    idx_cols = T * K_tot * 8
    idx16 = np.zeros((N_CORES, 16, idx_cols), dtype=np.int16)
    blk_base = e_t_in_core * K_tot * 8 + np.where(toA, 0, K_A * 8)
    idx16[e_core, e_j % 16, blk_base + e_j // 16] = e_val.astype(np.int16)
    idxT = np.tile(idx16, (1, P // 16, 1))

    S = np.zeros((N_CORES, P, T * KC * P), dtype=np.float32)
    S[e_core, e_p, e_col * P + e_dslot] = e_norm
    # self chunk: S[p, d] = (p == d) * dinv^2 of the node at (tile, d)
    n_core = (tile_of // T).astype(np.int64)
    n_t_in_core = tile_of % T
    n_slot = pos_of
    S[n_core, n_slot, (n_t_in_core * KC + K_tot) * P + n_slot] = norm_self

    # --- per-core transposed node features ---------------------------
    node_col = n_t_in_core * P + n_slot
    IN_pad = -(-IN // P) * P
    # full permuted features (same for every core) + per-core local rows
    xf = np.zeros((NG, IN_pad), dtype=np.float32)
    xf[row_of, :IN] = x
    xloc = xf.reshape(N_CORES, NPC, IN_pad)

    meta = dict(
        N=N, IN=IN, IN_pad=IN_pad, T=T, K_A=K_A, K_B=K_B, K=K_tot,
        NPC=NPC, NG=NG, WA=WA, WB_off=WB_off,
        node_core=n_core, node_col=node_col,
    )
    return xf, xloc, idxT, S, meta


def _assemble(outs, meta, OUT):
    """Gather per-core outputs back to the original node order."""
    N = meta["N"]
    full = np.empty((N, OUT), dtype=np.float32)
    node_core = meta["node_core"]
    node_col = meta["node_col"]
    for c in range(N_CORES):
        m = node_core == c
        full[m] = outs[c][node_col[m]]
    return full


# -------------------------------------------------------------- device side


def _build_program(T, K_A, K_B, KI, HID, OUT, NPC, NG, WA, WB_off, n_cores):
    import concourse.bacc as bacc
    import concourse.tile as tile
    import concourse.bass as bass
    from concourse import mybir
    from concourse.masks import make_identity

    f32 = mybir.dt.float32
    fmm = mybir.dt.float32r if USE_F32R else f32
    i16 = mybir.dt.int16
    K = K_A + K_B
    KC = K + 1
    IN_pad = KI * P
    KH = HID // P  # 128-chunks of hidden dim
    Relu = mybir.ActivationFunctionType.Relu

    nc = bacc.Bacc(
        "TRN2", target_bir_lowering=False, debug=False, num_devices=n_cores
    )

    xf = nc.dram_tensor("xf", [NG, IN_pad], fmm, kind="ExternalInput").ap()
    xl = nc.dram_tensor("xl", [NPC, IN_pad], fmm, kind="ExternalInput").ap()
    w1 = nc.dram_tensor("w1", [IN_pad, HID], f32, kind="ExternalInput").ap()
    b1 = nc.dram_tensor("b1", [1, HID], f32, kind="ExternalInput").ap()
    w2 = nc.dram_tensor("w2", [HID, OUT], f32, kind="ExternalInput").ap()
    b2 = nc.dram_tensor("b2", [1, OUT], f32, kind="ExternalInput").ap()
    s_in = nc.dram_tensor("s", [P, T * KC * P], fmm, kind="ExternalInput").ap()
    idxt = nc.dram_tensor("idxt", [P, T * K * 8], i16, kind="ExternalInput").ap()
    out = nc.dram_tensor("out", [NPC, OUT], f32, kind="ExternalOutput").ap()

    rg = [list(range(n_cores))]

    with tile.TileContext(nc) as tc:
        with (
            tc.tile_pool(name="dram", bufs=1, space="DRAM") as dpool,
            tc.tile_pool(name="const", bufs=1) as cpool,
            tc.tile_pool(name="work", bufs=3) as wpool,
            tc.tile_pool(name="gath", bufs=2) as gpool,
            tc.tile_pool(name="pers", bufs=1) as ppool,
            tc.tile_pool(name="ps", bufs=2, space="PSUM") as pspool,
        ):
            h2_loc = dpool.tile([NPC, OUT], f32)
            h2_full = dpool.tile([NG, OUT], f32, addr_space="Shared")

            # ---- constants -------------------------------------------------
            w1_sb = cpool.tile([P, KI * HID], f32)
            for ki in range(KI):
                nc.sync.dma_start(
                    out=w1_sb[:, ki * HID:(ki + 1) * HID],
                    in_=w1[ki * P:(ki + 1) * P, :],
                )
            w2_sb = cpool.tile([P, KH * OUT], f32)
            for kh in range(KH):
                nc.sync.dma_start(
                    out=w2_sb[:, kh * OUT:(kh + 1) * OUT],
                    in_=w2[kh * P:(kh + 1) * P, :],
                )
            b1_sb = cpool.tile([1, HID], f32)
            nc.sync.dma_start(out=b1_sb[:], in_=b1[:])
            b2_sb = cpool.tile([1, OUT], f32)
            nc.sync.dma_start(out=b2_sb[:], in_=b2[:])
            ones1 = cpool.tile([1, P], f32)
            nc.gpsimd.memset(ones1[:], 1.0)
            ident = cpool.tile([P, P], f32)
            make_identity(nc, ident[:])
            idx_sb = cpool.tile([P, T * K * 8], i16)
            nc.sync.dma_start(out=idx_sb[:], in_=idxt[:])

            aT = ppool.tile([P, KH * NPC], f32)  # transposed activations

            def gathers(t, h_full, h_loc, F, tag):
                """Windowed dma_gathers + self-chunk DMA for dst tile t;
                returns chunk k -> gathered [128, F] slice (k == K: self)."""
                blk = t * K * 8
                gA = gpool.tile([P, max(K_A, 1) * F], fmm, tag=tag + "A")
                if K_A > 0:
                    nc.gpsimd.dma_gather(
                        out_ap=gA[:].rearrange("p (k e) -> p k e", e=F),
                        in_ap=h_full[0:WA, :].bitcast(fmm),
                        idxs_ap=idx_sb[:, blk:blk + K_A * 8],
                        num_idxs=K_A * P,
                        num_idxs_reg=K_A * P,
                        elem_size=F,
                        single_packet=False,
                    )
                gB = None
                if K_B > 0:
                    gB = gpool.tile([P, K_B * F], fmm, tag=tag + "B")
                    nc.gpsimd.dma_gather(
                        out_ap=gB[:].rearrange("p (k e) -> p k e", e=F),
                        in_ap=h_full[WB_off:NG, :].bitcast(fmm),
                        idxs_ap=idx_sb[:, blk + K_A * 8:blk + K * 8],
                        num_idxs=K_B * P,
                        num_idxs_reg=K_B * P,
                        elem_size=F,
                        single_packet=False,
                    )
                gS = gpool.tile([P, F], fmm, tag=tag + "S")
                nc.sync.dma_start(
                    out=gS[:], in_=h_loc[t * P:(t + 1) * P, :].bitcast(fmm)
                )

                def chunk(k):
                    if k < K_A:
                        return gA[:, k * F:(k + 1) * F]
                    if k < K:
                        return gB[:, (k - K_A) * F:(k - K_A + 1) * F]
                    return gS[:]

                return chunk

            # ---- layer 1: aggx = S^T @ x[idx]; a = relu(aggx@W1 + b1) --
            for t in range(T):
                chunk = gathers(t, xf, xl, IN_pad, "g1")
                s_sb = gpool.tile([P, KC * P], fmm, tag="s1")
                nc.sync.dma_start(
                    out=s_sb[:], in_=s_in[:, t * KC * P:(t + 1) * KC * P]
                )
                psx = pspool.tile([P, IN_pad], f32, tag="ps_x")
                for k in range(KC):
                    nc.tensor.matmul(
                        psx[:],
                        lhsT=s_sb[:, k * P:(k + 1) * P],
                        rhs=chunk(k),
                        start=(k == 0),
                        stop=(k == KC - 1),
                    )
                agx = wpool.tile([P, IN_pad], f32, tag="agx")
                nc.vector.tensor_copy(out=agx[:], in_=psx[:])
                axT = wpool.tile([P, IN_pad], f32, tag="axT")
                for ki in range(KI):
                    pst = pspool.tile([P, P], f32, tag="ps_t")
                    nc.tensor.transpose(
                        out=pst[:],
                        in_=agx[:, ki * P:(ki + 1) * P],
                        identity=ident[:],
                    )
                    nc.vector.tensor_copy(
                        out=axT[:, ki * P:(ki + 1) * P], in_=pst[:]
                    )
                ps = pspool.tile([P, HID], f32, tag="ps_h")
                nc.tensor.matmul(
                    ps[:], lhsT=ones1[:], rhs=b1_sb[:], start=True, stop=False
                )
                for ki in range(KI):
                    nc.tensor.matmul(
                        ps[:],
                        lhsT=axT[:, ki * P:(ki + 1) * P],
                        rhs=w1_sb[:, ki * HID:(ki + 1) * HID],
                        start=False,
                        stop=(ki == KI - 1),
                    )
                a_t = wpool.tile([P, HID], f32, tag="a")
                nc.scalar.activation(out=a_t[:], in_=ps[:], func=Relu)
                for kh in range(KH):
                    pst = pspool.tile([P, P], f32, tag="ps_t")
                    nc.tensor.transpose(
                        out=pst[:],
                        in_=a_t[:, kh * P:(kh + 1) * P],
                        identity=ident[:],
                    )
                    nc.vector.tensor_copy(
                        out=aT[:, kh * NPC + t * P: kh * NPC + (t + 1) * P],
                        in_=pst[:],
                    )

            # ---- phase 4: h2 = a @ W2 -------------------------------------
            for t in range(T):
                ps = pspool.tile([P, OUT], f32, tag="ps_o")
                for kh in range(KH):
                    nc.tensor.matmul(
                        ps[:],
                        lhsT=aT[:, kh * NPC + t * P: kh * NPC + (t + 1) * P],
                        rhs=w2_sb[:, kh * OUT:(kh + 1) * OUT],
                        start=(kh == 0),
                        stop=(kh == KH - 1),
                    )
                h2t = wpool.tile([P, OUT], f32, tag="h2t")
                nc.vector.tensor_copy(out=h2t[:], in_=ps[:])
                nc.sync.dma_start(out=h2_loc[t * P:(t + 1) * P, :], in_=h2t[:])

            # ---- phase 5: AllGather h2 ------------------------------------
            nc.gpsimd.collective_compute(
                "AllGather",
                mybir.AluOpType.bypass,
                replica_groups=rg,
                ins=[h2_loc.opt()],
                outs=[h2_full.opt()],
            )

            # ---- phase 6: out = S^T @ h2[idx] + b2 ------------------------
            for t in range(T):
                chunk = gathers(t, h2_full, h2_loc, OUT, "g2")
                s_sb = gpool.tile([P, KC * P], fmm, tag="s1")
                nc.sync.dma_start(
                    out=s_sb[:], in_=s_in[:, t * KC * P:(t + 1) * KC * P]
                )
                ps = pspool.tile([P, OUT], f32, tag="ps_o")
                nc.tensor.matmul(
                    ps[:], lhsT=ones1[:], rhs=b2_sb[:], start=True, stop=False
                )
                for k in range(KC):
                    nc.tensor.matmul(
                        ps[:],
                        lhsT=s_sb[:, k * P:(k + 1) * P],
                        rhs=chunk(k),
                        start=False,
                        stop=(k == KC - 1),
                    )
                ot = wpool.tile([P, OUT], f32, tag="ot")
                nc.vector.tensor_copy(out=ot[:], in_=ps[:])
                nc.sync.dma_start(out=out[t * P:(t + 1) * P, :], in_=ot[:])

    nc.compile()
    return nc


def _get_program(T, K_A, K_B, KI, HID, OUT, NPC, NG, WA, WB_off,
                 n_cores=N_CORES):
    key = (T, K_A, K_B, KI, HID, OUT, NPC, NG, WA, WB_off, n_cores, USE_F32R)
    if key not in _prog_cache:
        _prog_cache[key] = _build_program(
            T, K_A, K_B, KI, HID, OUT, NPC, NG, WA, WB_off, n_cores
        )
    return _prog_cache[key]


# ------------------------------------------------------------------- driver


def _make_in_maps(x, edge_index, W1, b1, W2, b2):
    W1 = np.ascontiguousarray(np.asarray(W1, dtype=np.float32))
    W2 = np.ascontiguousarray(np.asarray(W2, dtype=np.float32))
    b1 = np.ascontiguousarray(np.asarray(b1, dtype=np.float32)).reshape(1, -1)
    b2 = np.ascontiguousarray(np.asarray(b2, dtype=np.float32)).reshape(1, -1)
    xf, xloc, idxT, S, meta = _preprocess(x, edge_index)
    IN_pad = meta["IN_pad"]
    HID = W1.shape[1]
    OUT = W2.shape[1]
    if W1.shape[0] < IN_pad:
        W1 = np.concatenate(
            [W1, np.zeros((IN_pad - W1.shape[0], HID), np.float32)], axis=0
        )
    in_maps = [
        {
            "xf": xf,
            "xl": xloc[c],
            "w1": W1,
            "b1": b1,
            "w2": W2,
            "b2": b2,
            "s": S[c],
            "idxt": idxT[c],
        }
        for c in range(N_CORES)
    ]
    return in_maps, meta, HID, OUT


def run(x, edge_index, W1, b1, W2, b2, trace=False, trace_cores=None):
    from concourse.bass_utils import run_bass_kernel_spmd

    in_maps, meta, HID, OUT = _make_in_maps(x, edge_index, W1, b1, W2, b2)
    nc = _get_program(
        meta["T"], meta["K_A"], meta["K_B"], meta["IN_pad"] // P, HID, OUT,
        meta["NPC"], meta["NG"], meta["WA"], meta["WB_off"],
    )
    res = run_bass_kernel_spmd(
        nc,
        in_maps,
        core_ids=list(range(N_CORES)),
        trace=trace,
        trace_cores=trace_cores,
    )
    outs = [res.results[c]["out"] for c in range(N_CORES)]
    return _assemble(outs, meta, OUT), res


def kernel(x, edge_index, W1, b1, W2, b2):
    full, _ = run(x, edge_index, W1, b1, W2, b2, trace=False)
    return full



# revision 5
# speedup vs baseline: 2.3190x; 2.3190x over previous
"""Two-layer GCN (PyG GCNConv-style) on 8 Trainium2 NeuronCores.

Strategy: nodes are partitioned across the 8 cores (load-balanced into
128-row destination tiles by in-degree), edges partitioned by destination
node so the segment-sum is local to the destination's core.

Both layers are transform-first (linearity of the GCN aggregation):
  layer 1:  h1 = x @ W1 (local rows)  -> AllGather -> aggregate
  layer 2:  h2 = a @ W2 (local rows)  -> AllGather -> aggregate
The symmetric norm dinv[s]*dinv[d] is split: dinv[src] is folded into the
gathered tables (h1s = dinv*h1, h2s = dinv*h2), dinv[dst] is applied on
the aggregated PSUM via per-partition activation scales (relu commutes
with the positive scale, so layer 1's dst factor rides into the h2 write
as dinv^2).  The per-chunk scatter matrix S is then a pure one-hot
matrix, generated on-device by the Vector engine as
S[e, d] = (iota[d] == dslot[e]) - no S traffic from HBM.  Self loops use
the identity matrix against the SBUF-resident local feature tiles.

Layer-1 aggregation is computed TRANSPOSED (aggT = chunk^T @ S) so the
relu'd result is directly the lhsT of the layer-2 GEMM - no transposes.

dma_gather descriptor generation costs ~8ns/row of GpSimd (Q7) time and
is the fundamental bottleneck (2x100k gathered rows per core).  The
ucode assigns each SWDGE queue to its own Q7 core pair, so gathers issued
round-robin on 4 queues generate descriptors 4x in parallel (measured).

Gathered tables, weights and matmul operands are fp16; PSUM accumulates
fp32.  dma_gather indices are int16, so the 50176-row tables are
addressed through two overlapping 32512-row windows.
"""

import numpy as np

P = 128
N_CORES = 8
WINDOW_CAP = 32512  # dma_gather int16 window (multiple of 128, <= 32767)
N_QUEUES = 4

_prog_cache = {}


# ---------------------------------------------------------------- host side


def _preprocess(x, edge_index):
    """Partition nodes/edges, build per-core device arrays."""
    x = np.asarray(x, dtype=np.float32)
    ei = np.asarray(edge_index)
    N, IN = x.shape

    src = ei[0].astype(np.int64)
    dst = ei[1].astype(np.int64)

    deg = 1 + np.bincount(dst, minlength=N)  # with self loop, >= 1
    dinv = (1.0 / np.sqrt(deg.astype(np.float64))).astype(np.float32)
    sqdeg = np.sqrt(deg.astype(np.float64)).astype(np.float32)

    npc_nodes = -(-N // N_CORES)
    T = -(-npc_nodes // P)  # dst tiles per core
    NPC = T * P  # node slots per core
    n_tiles = N_CORES * T
    NG = n_tiles * P  # global node slots

    # --- pack nodes into tiles, balancing per-tile in-degree (LPT) ----
    import heapq

    degg = deg - 1  # gathered (non-self) in-degree
    tile_of = np.empty(N, dtype=np.int64)
    pos_of = np.empty(N, dtype=np.int64)
    counts = np.zeros(n_tiles, dtype=np.int64)
    loads = np.zeros(n_tiles, dtype=np.int64)
    order = np.argsort(-degg, kind="stable")
    heap = [(0, t) for t in range(n_tiles)]
    heapq.heapify(heap)
    deg_l = degg[order]
    for i in range(N):
        v = order[i]
        while True:
            load, t = heapq.heappop(heap)
            if counts[t] < P:
                break
        tile_of[v] = t
        pos_of[v] = counts[t]
        counts[t] += 1
        load += int(deg_l[i])
        loads[t] = load
        if counts[t] < P:
            heapq.heappush(heap, (load, t))

    # repair pass: move small nodes off overloaded tiles toward the ideal
    # chunk count
    K_ideal = max(1, int(-(-int(degg.sum()) // (n_tiles * P))))
    target = K_ideal * P
    if loads.max() > target:
        by_tile = [[] for _ in range(n_tiles)]
        for i in range(N - 1, -1, -1):  # ascending degree order
            by_tile[tile_of[order[i]]].append(order[i])
        free = [(loads[t], t) for t in range(n_tiles)
                if counts[t] < P and loads[t] < target]
        heapq.heapify(free)
        for t_over in np.flatnonzero(loads > target):
            stack = by_tile[t_over]
            si = 0
            while loads[t_over] > target and si < len(stack) and free:
                v = stack[si]
                si += 1
                d = int(degg[v])
                moved = False
                tried = []
                while free:
                    lo, t2 = heapq.heappop(free)
                    if lo != loads[t2] or counts[t2] >= P:
                        continue  # stale
                    if loads[t2] + d <= target:
                        tile_of[v] = t2
                        pos_of[v] = counts[t2]
                        counts[t2] += 1
                        loads[t2] += d
                        loads[t_over] -= d
                        moved = True
                        if counts[t2] < P and loads[t2] < target:
                            heapq.heappush(free, (loads[t2], t2))
                        break
                    tried.append((lo, t2))
                for it in tried:
                    heapq.heappush(free, it)
                if not moved:
                    break
        # recompute pos_of consistently (holes possible after moves)
        ordv = np.lexsort((np.arange(N), tile_of))
        pos = np.empty(N, dtype=np.int64)
        tt = tile_of[ordv]
        st = np.zeros(n_tiles + 1, dtype=np.int64)
        np.cumsum(np.bincount(tt, minlength=n_tiles), out=st[1:])
        pos[ordv] = np.arange(N) - st[tt]
        pos_of = pos

    K = max(1, int(-(-loads.max() // P)))  # min gather chunks per dst tile

    row_of = tile_of * P + pos_of  # global new row of each node

    # --- per-edge placement (non-self edges) --------------------------
    e_tile = tile_of[dst]
    e_srcrow = row_of[src]

    sort_idx = np.lexsort((e_srcrow, e_tile))
    e_tile = e_tile[sort_idx]
    e_dslot = pos_of[dst][sort_idx].astype(np.int64)
    e_srcrow = e_srcrow[sort_idx]
    nE = len(e_tile)

    # --- window split (dma_gather int16 limit) ------------------------
    WA = min(WINDOW_CAP, NG)  # window A = rows [0, WA)
    WB_off = max(NG - WINDOW_CAP, 0)  # window B = rows [WB_off, NG)
    use_B = WB_off > 0

    tile_n = np.bincount(e_tile, minlength=n_tiles)
    if use_B:
        mustA = e_srcrow < WB_off
        mustB = e_srcrow >= WA
        flex = ~mustA & ~mustB
        cntA = np.bincount(e_tile[mustA], minlength=n_tiles)
        cntB = np.bincount(e_tile[mustB], minlength=n_tiles)
        found = None
        K_tot = K
        while found is None:
            mid = -(-K_tot // 2)
            for d in range(K_tot + 1):
                for K_A in {mid + d, mid - d}:
                    if not 0 <= K_A <= K_tot:
                        continue
                    K_B = K_tot - K_A
                    if (
                        cntA.max() <= K_A * P
                        and cntB.max() <= K_B * P
                        and tile_n.max() <= (K_A + K_B) * P
                    ):
                        found = (K_A, K_B)
                        break
                if found:
                    break
            if not found:
                K_tot += 1
        K_A, K_B = found
        capB = K_B * P
        nA_t = np.minimum(K_A * P, cntA + np.bincount(
            e_tile[flex], minlength=n_tiles))
        nA_t = np.maximum(nA_t, tile_n - capB)
        flexA_quota = nA_t - cntA
        flex_idx = np.flatnonzero(flex)
        ft = e_tile[flex_idx]
        fstart = np.zeros(n_tiles + 1, dtype=np.int64)
        np.cumsum(np.bincount(ft, minlength=n_tiles), out=fstart[1:])
        frank = np.arange(len(ft)) - fstart[ft]
        toA = mustA.copy()
        toA[flex_idx[frank < flexA_quota[ft]]] = True
    else:
        K_A, K_B = K, 0
        toA = np.ones(nE, dtype=bool)
    K_tot = K_A + K_B

    # --- slot assignment within each (tile, window) -------------------
    e_j = np.empty(nE, dtype=np.int64)  # position within its window list
    e_val = np.empty(nE, dtype=np.int64)  # int16 index value
    for is_A in (True, False):
        m = toA if is_A else ~toA
        if not m.any():
            continue
        idxs = np.flatnonzero(m)
        t_sel = e_tile[idxs]
        start = np.zeros(n_tiles + 1, dtype=np.int64)
        np.cumsum(np.bincount(t_sel, minlength=n_tiles), out=start[1:])
        e_j[idxs] = np.arange(len(idxs)) - start[t_sel]
        e_val[idxs] = e_srcrow[idxs] - (0 if is_A else WB_off)

    e_p = e_j % P  # partition (edge slot)
    e_chunk = np.where(toA, e_j // P, K_A + e_j // P)  # chunk within tile

    e_core = e_tile // T
    e_t_in_core = e_tile % T

    # --- idx tables, tile-major: value j at [j%16, t*Kw*8 + j//16] ----
    idxA = np.zeros((N_CORES, 16, T * max(K_A, 1) * 8), dtype=np.int16)
    idxB = np.zeros((N_CORES, 16, T * max(K_B, 1) * 8), dtype=np.int16)
    for arr, sel, Kw in ((idxA, toA, K_A), (idxB, ~toA, K_B)):
        if Kw == 0:
            continue
        m = np.flatnonzero(sel)
        arr[e_core[m], e_j[m] % 16, e_t_in_core[m] * Kw * 8 + e_j[m] // 16] = (
            e_val[m].astype(np.int16)
        )
    idxA = np.tile(idxA, (1, 8, 1))  # [cores, 128, T*K_A*8]
    idxB = np.tile(idxB, (1, 8, 1))

    # --- dslot table: [core, 128, T*K_tot] fp16, padding -1 -----------
    scm = np.full((N_CORES, P, T * K_tot), -1.0, dtype=np.float16)
    scm[e_core, e_p, e_t_in_core * K_tot + e_chunk] = e_dslot.astype(np.float16)

    # --- per-node scale vectors, per core -----------------------------
    n_core = (tile_of // T).astype(np.int64)
    n_t_in_core = tile_of % T
    n_slot = pos_of
    dinv_col = np.zeros((N_CORES, P, T), dtype=np.float32)
    dinv2_col = np.zeros((N_CORES, P, T), dtype=np.float32)
    sqdeg_row = np.zeros((N_CORES, 1, NPC), dtype=np.float16)
    dinv_col[n_core, n_slot, n_t_in_core] = dinv
    dinv2_col[n_core, n_slot, n_t_in_core] = dinv * dinv
    sqdeg_row[n_core, 0, n_t_in_core * P + n_slot] = sqdeg.astype(np.float16)

    # --- per-core transposed node features, fp16, tile-major ----------
    KI = -(-IN // P)
    IN_pad = KI * P
    xf16 = x.astype(np.float16)
    xlT = np.zeros((N_CORES, P, T * IN_pad), dtype=np.float16)
    for ki in range(KI):
        pp = min(P, IN - ki * P)
        cols = n_t_in_core * IN_pad + ki * P + n_slot
        xlT[n_core, :pp, cols] = xf16[:, ki * P:ki * P + pp]

    meta = dict(
        N=N, IN=IN, IN_pad=IN_pad, KI=KI, T=T, K_A=K_A, K_B=K_B, K=K_tot,
        NPC=NPC, NG=NG, WA=WA, WB_off=WB_off,
        node_core=n_core, node_col=n_t_in_core * P + n_slot,
    )
    arrs = dict(
        xlT=xlT, idxA=idxA, idxB=idxB, scm=scm,
        dinv_col=dinv_col, dinv2_col=dinv2_col, sqdeg_row=sqdeg_row,
    )
    return arrs, meta


def _assemble(outs, meta, OUT):
    """Gather per-core outputs back to the original node order."""
    N = meta["N"]
    full = np.empty((N, OUT), dtype=np.float32)
    node_core = meta["node_core"]
    node_col = meta["node_col"]
    for c in range(N_CORES):
        m = node_core == c
        full[m] = outs[c][node_col[m]]
    return full


# -------------------------------------------------------------- device side


def _build_program(T, K_A, K_B, KI, HID, OUT, NPC, NG, WA, WB_off,
                   has_b1, has_b2, n_cores):
    import concourse.bacc as bacc
    import concourse.tile as tile
    from concourse import mybir
    from concourse.masks import make_identity

    f32 = mybir.dt.float32
    f16 = mybir.dt.float16
    i16 = mybir.dt.int16
    K = K_A + K_B
    IN_pad = KI * P
    KH = HID // P  # 128-chunks of hidden dim
    Relu = mybir.ActivationFunctionType.Relu
    Copy = mybir.ActivationFunctionType.Copy

    nc = bacc.Bacc(
        "TRN2", target_bir_lowering=False, debug=False, num_devices=n_cores,
        num_swdge_queues=N_QUEUES,
    )

    xlT = nc.dram_tensor("xlT", [P, T * IN_pad], f16, kind="ExternalInput").ap()
    w1 = nc.dram_tensor("w1", [P, KI * HID], f16, kind="ExternalInput").ap()
    b1 = nc.dram_tensor("b1", [1, HID], f16, kind="ExternalInput").ap()
    w2 = nc.dram_tensor("w2", [P, KH * OUT], f16, kind="ExternalInput").ap()
    b2 = nc.dram_tensor("b2", [1, OUT], f16, kind="ExternalInput").ap()
    idxA_d = nc.dram_tensor(
        "idxA", [P, T * max(K_A, 1) * 8], i16, kind="ExternalInput").ap()
    idxB_d = nc.dram_tensor(
        "idxB", [P, T * max(K_B, 1) * 8], i16, kind="ExternalInput").ap()
    scm_d = nc.dram_tensor("scm", [P, T * K], f16, kind="ExternalInput").ap()
    dinv_d = nc.dram_tensor("dinv", [P, T], f32, kind="ExternalInput").ap()
    dinv2_d = nc.dram_tensor("dinv2", [P, T], f32, kind="ExternalInput").ap()
    sqdeg_d = nc.dram_tensor("sqdeg", [1, NPC], f16, kind="ExternalInput").ap()
    out = nc.dram_tensor("out", [NPC, OUT], f32, kind="ExternalOutput").ap()

    rg = [list(range(n_cores))]

    with tile.TileContext(nc) as tc:
        with (
            tc.tile_pool(name="dram", bufs=1, space="DRAM") as dpool,
            tc.tile_pool(name="const", bufs=1) as cpool,
            tc.tile_pool(name="pers", bufs=1) as ppool,
            tc.tile_pool(name="work", bufs=3) as wpool,
            tc.tile_pool(name="gath", bufs=6) as gpool,
            tc.tile_pool(name="sgen", bufs=4) as spool,
            tc.tile_pool(name="ps", bufs=2, space="PSUM") as pspool,
            tc.tile_pool(name="psg", bufs=2, space="PSUM") as psgpool,
        ):
            h1s_loc = dpool.tile([NPC, HID], f16)
            h1s_full = dpool.tile([NG, HID], f16, addr_space="Shared")
            h2s_loc = dpool.tile([NPC, OUT], f16)
            h2s_full = dpool.tile([NG, OUT], f16, addr_space="Shared")

            # ---- constants -------------------------------------------------
            w1_sb = cpool.tile([P, KI * HID], f16)
            nc.sync.dma_start(out=w1_sb[:], in_=w1[:])
            w2_sb = cpool.tile([P, KH * OUT], f16)
            nc.sync.dma_start(out=w2_sb[:], in_=w2[:])
            b1_sb = cpool.tile([1, HID], f16)
            nc.sync.dma_start(out=b1_sb[:], in_=b1[:])
            b2_sb = cpool.tile([1, OUT], f16)
            nc.sync.dma_start(out=b2_sb[:], in_=b2[:])
            ident = cpool.tile([P, P], f16)
            make_identity(nc, ident[:])
            idxA_sb = cpool.tile([P, T * max(K_A, 1) * 8], i16)
            nc.sync.dma_start(out=idxA_sb[:], in_=idxA_d[:])
            idxB_sb = cpool.tile([P, T * max(K_B, 1) * 8], i16)
            nc.sync.dma_start(out=idxB_sb[:], in_=idxB_d[:])
            scm_sb = cpool.tile([P, T * K], f16)
            nc.sync.dma_start(out=scm_sb[:], in_=scm_d[:])
            dinv_sb = cpool.tile([P, T], f32)
            nc.sync.dma_start(out=dinv_sb[:], in_=dinv_d[:])
            dinv2_sb = cpool.tile([P, T], f32)
            nc.sync.dma_start(out=dinv2_sb[:], in_=dinv2_d[:])
            sqdeg_sb = cpool.tile([1, NPC], f16)
            nc.sync.dma_start(out=sqdeg_sb[:], in_=sqdeg_d[:])
            # iota[p, d] = d  (fp16-exact for d < 2048)
            iota_i = cpool.tile([P, P], i16)
            nc.gpsimd.iota(iota_i[:], pattern=[[1, P]], base=0,
                           channel_multiplier=0)
            iota_sb = cpool.tile([P, P], f16)
            nc.vector.tensor_copy(out=iota_sb[:], in_=iota_i[:])

            h1s_sb = ppool.tile([P, T * HID], f16)  # local scaled h1 tiles
            h2s_sb = ppool.tile([P, T * OUT], f16)  # local scaled h2 tiles

            # ---- phase A: h1s = dinv * (x @ W1), local rows ---------------
            for t in range(T):
                xt = wpool.tile([P, IN_pad], f16, tag="xt")
                nc.sync.dma_start(
                    out=xt[:], in_=xlT[:, t * IN_pad:(t + 1) * IN_pad]
                )
                ps = pspool.tile([P, HID], f32, tag="ps")
                for ki in range(KI):
                    nc.tensor.matmul(
                        ps[:],
                        lhsT=xt[:, ki * P:(ki + 1) * P],
                        rhs=w1_sb[:, ki * HID:(ki + 1) * HID],
                        start=(ki == 0),
                        stop=(ki == KI - 1),
                    )
                h1t = h1s_sb[:, t * HID:(t + 1) * HID]
                nc.scalar.activation(
                    out=h1t, in_=ps[:], func=Copy,
                    scale=dinv_sb[:, t:t + 1],
                )
                nc.sync.dma_start(
                    out=h1s_loc[t * P:(t + 1) * P, :], in_=h1t
                )

            # ---- phase B: AllGather h1s -----------------------------------
            nc.gpsimd.collective_compute(
                "AllGather",
                mybir.AluOpType.bypass,
                replica_groups=rg,
                ins=[h1s_loc.opt()],
                outs=[h1s_full.opt()],
            )

            def gathers(t, h_full, F):
                """Windowed dma_gathers for dst tile t on rotating queues;
                returns k -> gathered [128, F] slice."""
                gA = gpool.tile([P, max(K_A, 1) * 256], f16, tag="gA",
                                name="gA")
                gB = gpool.tile([P, max(K_B, 1) * 256], f16, tag="gB",
                                name="gB")
                if K_A > 0:
                    nc.gpsimd.dma_gather(
                        out_ap=gA[:, :K_A * F].rearrange(
                            "p (k e) -> p k e", e=F),
                        in_ap=h_full[0:WA, :],
                        idxs_ap=idxA_sb[:, t * K_A * 8:(t + 1) * K_A * 8],
                        num_idxs=K_A * P,
                        num_idxs_reg=K_A * P,
                        elem_size=F,
                        single_packet=False,
                        queue_num=t % N_QUEUES,
                    )
                if K_B > 0:
                    nc.gpsimd.dma_gather(
                        out_ap=gB[:, :K_B * F].rearrange(
                            "p (k e) -> p k e", e=F),
                        in_ap=h_full[WB_off:NG, :],
                        idxs_ap=idxB_sb[:, t * K_B * 8:(t + 1) * K_B * 8],
                        num_idxs=K_B * P,
                        num_idxs_reg=K_B * P,
                        elem_size=F,
                        single_packet=False,
                        queue_num=t % N_QUEUES,
                    )

                def chunk(k):
                    if k < K_A:
                        return gA[:, k * F:(k + 1) * F]
                    j = k - K_A
                    return gB[:, j * F:(j + 1) * F]

                return chunk

            def gen_s(t):
                """One-hot scatter matrices for tile t: [128, K, 128] fp16."""
                s_sb = spool.tile([P, K, P], f16, tag="s", name="s_sb")
                nc.vector.tensor_tensor(
                    out=s_sb[:],
                    in0=iota_sb[:].rearrange("p (o d) -> p o d", o=1)
                        .broadcast_to([P, K, P]),
                    in1=scm_sb[:, t * K:(t + 1) * K]
                        .rearrange("p (k o) -> p k o", o=1)
                        .broadcast_to([P, K, P]),
                    op=mybir.AluOpType.is_equal,
                )
                return s_sb

            # ---- phase C: layer-1 aggregate (transposed) + GEMM2 ----------
            for t in range(T):
                chunk = gathers(t, h1s_full, HID)
                s_sb = gen_s(t)
                aT = wpool.tile([P, KH * P], f16, tag="aT")
                for kh in range(KH):
                    psT = psgpool.tile([P, P], f32, tag=f"psT{kh}")
                    first = True
                    if has_b1:
                        nc.tensor.matmul(
                            psT[:],
                            lhsT=b1_sb[:, kh * P:(kh + 1) * P],
                            rhs=sqdeg_sb[:, t * P:(t + 1) * P],
                            start=True, stop=False,
                        )
                        first = False
                    nc.tensor.matmul(
                        psT[:],
                        lhsT=h1s_sb[:, t * HID + kh * P:
                                    t * HID + (kh + 1) * P],
                        rhs=ident[:],
                        start=first, stop=False,
                    )
                    for k in range(K):
                        nc.tensor.matmul(
                            psT[:],
                            lhsT=chunk(k)[:, kh * P:(kh + 1) * P],
                            rhs=s_sb[:, k, :],
                            start=False, stop=(k == K - 1),
                        )
                    nc.scalar.activation(
                        out=aT[:, kh * P:(kh + 1) * P], in_=psT[:],
                        func=Relu,
                    )
                ps2_full = pspool.tile([P, HID], f32, tag="ps", name="ps2")
                ps2 = ps2_full[:, :OUT]
                for kh in range(KH):
                    nc.tensor.matmul(
                        ps2[:],
                        lhsT=aT[:, kh * P:(kh + 1) * P],
                        rhs=w2_sb[:, kh * OUT:(kh + 1) * OUT],
                        start=(kh == 0),
                        stop=(kh == KH - 1),
                    )
                h2t = h2s_sb[:, t * OUT:(t + 1) * OUT]
                nc.scalar.activation(
                    out=h2t, in_=ps2[:], func=Copy,
                    scale=dinv2_sb[:, t:t + 1],
                )
                nc.sync.dma_start(
                    out=h2s_loc[t * P:(t + 1) * P, :], in_=h2t
                )

            # ---- phase D: AllGather h2s -----------------------------------
            nc.gpsimd.collective_compute(
                "AllGather",
                mybir.AluOpType.bypass,
                replica_groups=rg,
                ins=[h2s_loc.opt()],
                outs=[h2s_full.opt()],
            )

            # ---- phase E: layer-2 aggregate -------------------------------
            for t in range(T):
                chunk = gathers(t, h2s_full, OUT)
                s_sb = gen_s(t)
                ps_full = pspool.tile([P, HID], f32, tag="ps", name="ps")
                ps = ps_full[:, :OUT]
                first = True
                if has_b2:
                    nc.tensor.matmul(
                        ps[:],
                        lhsT=sqdeg_sb[:, t * P:(t + 1) * P],
                        rhs=b2_sb[:],
                        start=True, stop=False,
                    )
                    first = False
                nc.tensor.matmul(
                    ps[:],
                    lhsT=ident[:],
                    rhs=h2s_sb[:, t * OUT:(t + 1) * OUT],
                    start=first, stop=False,
                )
                for k in range(K):
                    nc.tensor.matmul(
                        ps[:],
                        lhsT=s_sb[:, k, :],
                        rhs=chunk(k),
                        start=False, stop=(k == K - 1),
                    )
                ot = wpool.tile([P, OUT], f32, tag="ot")
                nc.scalar.activation(
                    out=ot[:], in_=ps[:], func=Copy,
                    scale=dinv_sb[:, t:t + 1],
                )
                nc.sync.dma_start(out=out[t * P:(t + 1) * P, :], in_=ot[:])

    nc.compile()
    return nc


def _get_program(T, K_A, K_B, KI, HID, OUT, NPC, NG, WA, WB_off,
                 has_b1, has_b2, n_cores=N_CORES):
    key = (T, K_A, K_B, KI, HID, OUT, NPC, NG, WA, WB_off,
           has_b1, has_b2, n_cores)
    if key not in _prog_cache:
        _prog_cache[key] = _build_program(
            T, K_A, K_B, KI, HID, OUT, NPC, NG, WA, WB_off,
            has_b1, has_b2, n_cores
        )
    return _prog_cache[key]


# ------------------------------------------------------------------- driver


def _make_in_maps(x, edge_index, W1, b1, W2, b2):
    W1 = np.asarray(W1, dtype=np.float32)
    W2 = np.asarray(W2, dtype=np.float32)
    b1 = np.asarray(b1, dtype=np.float32).reshape(1, -1)
    b2 = np.asarray(b2, dtype=np.float32).reshape(1, -1)
    arrs, meta = _preprocess(x, edge_index)
    IN_pad = meta["IN_pad"]
    KI = meta["KI"]
    HID = W1.shape[1]
    OUT = W2.shape[1]
    if W1.shape[0] < IN_pad:
        W1 = np.concatenate(
            [W1, np.zeros((IN_pad - W1.shape[0], HID), np.float32)], axis=0
        )
    # device layout: w1 [128, KI*HID] fp16 (chunk ki at cols ki*HID..)
    w1_dev = np.concatenate(
        [W1[ki * P:(ki + 1) * P].astype(np.float16) for ki in range(KI)],
        axis=1,
    )
    KH = HID // P
    w2_dev = np.concatenate(
        [W2[kh * P:(kh + 1) * P].astype(np.float16) for kh in range(KH)],
        axis=1,
    )
    in_maps = [
        {
            "xlT": arrs["xlT"][c],
            "w1": w1_dev,
            "b1": b1.astype(np.float16),
            "w2": w2_dev,
            "b2": b2.astype(np.float16),
            "idxA": arrs["idxA"][c],
            "idxB": arrs["idxB"][c],
            "scm": arrs["scm"][c],
            "dinv": arrs["dinv_col"][c],
            "dinv2": arrs["dinv2_col"][c],
            "sqdeg": arrs["sqdeg_row"][c],
        }
        for c in range(N_CORES)
    ]
    has_b1 = bool(np.any(b1 != 0))
    has_b2 = bool(np.any(b2 != 0))
    return in_maps, meta, HID, OUT, has_b1, has_b2


def run(x, edge_index, W1, b1, W2, b2, trace=False, trace_cores=None):
    from concourse.bass_utils import run_bass_kernel_spmd

    in_maps, meta, HID, OUT, has_b1, has_b2 = _make_in_maps(
        x, edge_index, W1, b1, W2, b2)
    nc = _get_program(
        meta["T"], meta["K_A"], meta["K_B"], meta["KI"], HID, OUT,
        meta["NPC"], meta["NG"], meta["WA"], meta["WB_off"],
        has_b1, has_b2,
    )
    res = run_bass_kernel_spmd(
        nc,
        in_maps,
        core_ids=list(range(N_CORES)),
        trace=trace,
        trace_cores=trace_cores,
    )
    outs = [res.results[c]["out"] for c in range(N_CORES)]
    return _assemble(outs, meta, OUT), res


def kernel(x, edge_index, W1, b1, W2, b2):
    full, _ = run(x, edge_index, W1, b1, W2, b2, trace=False)
    return full


# revision 11
# speedup vs baseline: 2.4585x; 1.0602x over previous
"""Two-layer GCN (PyG GCNConv-style) on 8 Trainium2 NeuronCores.

Strategy: nodes are partitioned across the 8 cores (load-balanced into
128-row destination tiles by in-degree), edges partitioned by destination
node so the segment-sum is local to the destination's core.

Both layers are transform-first (linearity of the GCN aggregation):
  layer 1:  h1 = x @ W1 (local rows)  -> AllGather -> aggregate
  layer 2:  h2 = a @ W2 (local rows)  -> AllGather -> aggregate
The symmetric norm dinv[s]*dinv[d] is split: dinv[src] is folded into the
gathered tables (h1s = dinv*h1, h2s = dinv*h2), dinv[dst] is applied on
the aggregated PSUM via per-partition activation scales (relu commutes
with the positive scale, so layer 1's dst factor rides into the h2 write
as dinv^2).  The per-chunk scatter matrix S is then a pure one-hot
matrix, generated on-device by the Vector engine as
S[e, d] = (iota[d] == dslot[e]) - no S traffic from HBM.  Self loops use
the identity matrix against the SBUF-resident local feature tiles.

Layer-1 aggregation is computed TRANSPOSED (aggT = chunk^T @ S) so the
relu'd result is directly the lhsT of the layer-2 GEMM - no transposes.

dma_gather descriptor generation costs ~8ns/row of GpSimd (Q7) time and
is the fundamental bottleneck (2x100k gathered rows per core).  The
ucode assigns each SWDGE queue to its own Q7 core pair, so gathers issued
round-robin on 4 queues generate descriptors 4x in parallel (measured).

Gathered tables, weights and matmul operands are fp16; PSUM accumulates
fp32.  dma_gather indices are int16, so the 50176-row tables are
addressed through two overlapping 32512-row windows.
"""

import numpy as np

P = 128
N_CORES = 8
WINDOW_CAP = 32512  # dma_gather int16 window (multiple of 128, <= 32767)
N_QUEUES = 4

_prog_cache = {}


# ---------------------------------------------------------------- host side


def _preprocess(x, edge_index):
    """Partition nodes/edges, build per-core device arrays."""
    x = np.asarray(x, dtype=np.float32)
    ei = np.asarray(edge_index)
    N, IN = x.shape

    src = ei[0].astype(np.int64)
    dst = ei[1].astype(np.int64)

    deg = 1 + np.bincount(dst, minlength=N)  # with self loop, >= 1
    dinv = (1.0 / np.sqrt(deg.astype(np.float64))).astype(np.float32)
    sqdeg = np.sqrt(deg.astype(np.float64)).astype(np.float32)

    npc_nodes = -(-N // N_CORES)
    T = -(-npc_nodes // P)  # dst tiles per core
    NPC = T * P  # node slots per core
    n_tiles = N_CORES * T
    NG = n_tiles * P  # global node slots

    # --- pack nodes into tiles, balancing per-tile in-degree (LPT) ----
    import heapq

    degg = deg - 1  # gathered (non-self) in-degree
    tile_of = np.empty(N, dtype=np.int64)
    pos_of = np.empty(N, dtype=np.int64)
    counts = np.zeros(n_tiles, dtype=np.int64)
    loads = np.zeros(n_tiles, dtype=np.int64)
    order = np.argsort(-degg, kind="stable")
    heap = [(0, t) for t in range(n_tiles)]
    heapq.heapify(heap)
    deg_l = degg[order]
    for i in range(N):
        v = order[i]
        while True:
            load, t = heapq.heappop(heap)
            if counts[t] < P:
                break
        tile_of[v] = t
        pos_of[v] = counts[t]
        counts[t] += 1
        load += int(deg_l[i])
        loads[t] = load
        if counts[t] < P:
            heapq.heappush(heap, (load, t))

    # repair pass: move small nodes off overloaded tiles toward the ideal
    # chunk count
    K_ideal = max(1, int(-(-int(degg.sum()) // (n_tiles * P))))
    target = K_ideal * P
    if loads.max() > target:
        by_tile = [[] for _ in range(n_tiles)]
        for i in range(N - 1, -1, -1):  # ascending degree order
            by_tile[tile_of[order[i]]].append(order[i])
        free = [(loads[t], t) for t in range(n_tiles)
                if counts[t] < P and loads[t] < target]
        heapq.heapify(free)
        for t_over in np.flatnonzero(loads > target):
            stack = by_tile[t_over]
            si = 0
            while loads[t_over] > target and si < len(stack) and free:
                v = stack[si]
                si += 1
                d = int(degg[v])
                moved = False
                tried = []
                while free:
                    lo, t2 = heapq.heappop(free)
                    if lo != loads[t2] or counts[t2] >= P:
                        continue  # stale
                    if loads[t2] + d <= target:
                        tile_of[v] = t2
                        pos_of[v] = counts[t2]
                        counts[t2] += 1
                        loads[t2] += d
                        loads[t_over] -= d
                        moved = True
                        if counts[t2] < P and loads[t2] < target:
                            heapq.heappush(free, (loads[t2], t2))
                        break
                    tried.append((lo, t2))
                for it in tried:
                    heapq.heappush(free, it)
                if not moved:
                    break
        # recompute pos_of consistently (holes possible after moves)
        ordv = np.lexsort((np.arange(N), tile_of))
        pos = np.empty(N, dtype=np.int64)
        tt = tile_of[ordv]
        st = np.zeros(n_tiles + 1, dtype=np.int64)
        np.cumsum(np.bincount(tt, minlength=n_tiles), out=st[1:])
        pos[ordv] = np.arange(N) - st[tt]
        pos_of = pos

    K = max(1, int(-(-loads.max() // P)))  # min gather chunks per dst tile

    row_of = tile_of * P + pos_of  # global new row of each node

    # --- per-edge placement (non-self edges) --------------------------
    e_tile = tile_of[dst]
    e_srcrow = row_of[src]

    sort_idx = np.lexsort((e_srcrow, e_tile))
    e_tile = e_tile[sort_idx]
    e_dslot = pos_of[dst][sort_idx].astype(np.int64)
    e_srcrow = e_srcrow[sort_idx]
    nE = len(e_tile)

    # --- window split (dma_gather int16 limit) ------------------------
    WA = min(WINDOW_CAP, NG)  # window A = rows [0, WA)
    WB_off = max(NG - WINDOW_CAP, 0)  # window B = rows [WB_off, NG)
    use_B = WB_off > 0

    tile_n = np.bincount(e_tile, minlength=n_tiles)
    if use_B:
        mustA = e_srcrow < WB_off
        mustB = e_srcrow >= WA
        flex = ~mustA & ~mustB
        cntA = np.bincount(e_tile[mustA], minlength=n_tiles)
        cntB = np.bincount(e_tile[mustB], minlength=n_tiles)
        found = None
        K_tot = K
        while found is None:
            mid = -(-K_tot // 2)
            for d in range(K_tot + 1):
                for K_A in {mid + d, mid - d}:
                    if not 0 <= K_A <= K_tot:
                        continue
                    K_B = K_tot - K_A
                    if (
                        cntA.max() <= K_A * P
                        and cntB.max() <= K_B * P
                        and tile_n.max() <= (K_A + K_B) * P
                    ):
                        found = (K_A, K_B)
                        break
                if found:
                    break
            if not found:
                K_tot += 1
        K_A, K_B = found
        capB = K_B * P
        nA_t = np.minimum(K_A * P, cntA + np.bincount(
            e_tile[flex], minlength=n_tiles))
        nA_t = np.maximum(nA_t, tile_n - capB)
        flexA_quota = nA_t - cntA
        flex_idx = np.flatnonzero(flex)
        ft = e_tile[flex_idx]
        fstart = np.zeros(n_tiles + 1, dtype=np.int64)
        np.cumsum(np.bincount(ft, minlength=n_tiles), out=fstart[1:])
        frank = np.arange(len(ft)) - fstart[ft]
        toA = mustA.copy()
        toA[flex_idx[frank < flexA_quota[ft]]] = True
    else:
        K_A, K_B = K, 0
        toA = np.ones(nE, dtype=bool)
    K_tot = K_A + K_B

    # --- slot assignment within each (tile, window) -------------------
    e_j = np.empty(nE, dtype=np.int64)  # position within its window list
    e_val = np.empty(nE, dtype=np.int64)  # int16 index value
    for is_A in (True, False):
        m = toA if is_A else ~toA
        if not m.any():
            continue
        idxs = np.flatnonzero(m)
        t_sel = e_tile[idxs]
        start = np.zeros(n_tiles + 1, dtype=np.int64)
        np.cumsum(np.bincount(t_sel, minlength=n_tiles), out=start[1:])
        e_j[idxs] = np.arange(len(idxs)) - start[t_sel]
        e_val[idxs] = e_srcrow[idxs] - (0 if is_A else WB_off)

    e_p = e_j % P  # partition (edge slot)
    e_chunk = np.where(toA, e_j // P, K_A + e_j // P)  # chunk within tile

    e_core = e_tile // T
    e_t_in_core = e_tile % T

    # --- idx tables, tile-major: value j at [j%16, t*Kw*8 + j//16] ----
    idxA = np.zeros((N_CORES, 16, T * max(K_A, 1) * 8), dtype=np.int16)
    idxB = np.zeros((N_CORES, 16, T * max(K_B, 1) * 8), dtype=np.int16)
    for arr, sel, Kw in ((idxA, toA, K_A), (idxB, ~toA, K_B)):
        if Kw == 0:
            continue
        m = np.flatnonzero(sel)
        arr[e_core[m], e_j[m] % 16, e_t_in_core[m] * Kw * 8 + e_j[m] // 16] = (
            e_val[m].astype(np.int16)
        )
    idxA = np.tile(idxA, (1, 8, 1))  # [cores, 128, T*K_A*8]
    idxB = np.tile(idxB, (1, 8, 1))

    # --- dslot table: [core, 128, T*K_tot] fp16, padding -1 -----------
    scm = np.full((N_CORES, P, T * K_tot), -1.0, dtype=np.float16)
    scm[e_core, e_p, e_t_in_core * K_tot + e_chunk] = e_dslot.astype(np.float16)

    # --- per-node scale vectors, per core -----------------------------
    n_core = (tile_of // T).astype(np.int64)
    n_t_in_core = tile_of % T
    n_slot = pos_of
    dinv_col = np.zeros((N_CORES, P, T), dtype=np.float32)
    dinv2_col = np.zeros((N_CORES, P, T), dtype=np.float32)
    sqdeg_row = np.zeros((N_CORES, 1, NPC), dtype=np.float16)
    dinv_col[n_core, n_slot, n_t_in_core] = dinv
    dinv2_col[n_core, n_slot, n_t_in_core] = dinv * dinv
    sqdeg_row[n_core, 0, n_t_in_core * P + n_slot] = sqdeg.astype(np.float16)

    # --- per-core transposed node features, fp16, tile-major ----------
    KI = -(-IN // P)
    IN_pad = KI * P
    xf16 = x.astype(np.float16)
    xlT = np.zeros((N_CORES, P, T * IN_pad), dtype=np.float16)
    for ki in range(KI):
        pp = min(P, IN - ki * P)
        cols = n_t_in_core * IN_pad + ki * P + n_slot
        xlT[n_core, :pp, cols] = xf16[:, ki * P:ki * P + pp]

    meta = dict(
        N=N, IN=IN, IN_pad=IN_pad, KI=KI, T=T, K_A=K_A, K_B=K_B, K=K_tot,
        NPC=NPC, NG=NG, WA=WA, WB_off=WB_off,
        node_core=n_core, node_col=n_t_in_core * P + n_slot,
    )
    arrs = dict(
        xlT=xlT, idxA=idxA, idxB=idxB, scm=scm,
        dinv_col=dinv_col, dinv2_col=dinv2_col, sqdeg_row=sqdeg_row,
    )
    return arrs, meta


def _assemble(outs, meta, OUT):
    """Gather per-core outputs back to the original node order."""
    N = meta["N"]
    full = np.empty((N, OUT), dtype=np.float32)
    node_core = meta["node_core"]
    node_col = meta["node_col"]
    for c in range(N_CORES):
        m = node_core == c
        full[m] = outs[c][node_col[m]]
    return full


# -------------------------------------------------------------- device side


def _build_program(T, K_A, K_B, KI, HID, OUT, NPC, NG, WA, WB_off,
                   has_b1, has_b2, n_cores):
    import concourse.bacc as bacc
    import concourse.tile as tile
    from concourse import mybir
    from concourse.masks import make_identity

    f32 = mybir.dt.float32
    f16 = mybir.dt.float16
    i16 = mybir.dt.int16
    K = K_A + K_B
    IN_pad = KI * P
    KH = HID // P  # 128-chunks of hidden dim
    Relu = mybir.ActivationFunctionType.Relu
    Copy = mybir.ActivationFunctionType.Copy

    nc = bacc.Bacc(
        "TRN2", target_bir_lowering=False, debug=False, num_devices=n_cores,
        num_swdge_queues=N_QUEUES,
    )

    xlT = nc.dram_tensor("xlT", [P, T * IN_pad], f16, kind="ExternalInput").ap()
    w1 = nc.dram_tensor("w1", [P, KI * HID], f16, kind="ExternalInput").ap()
    b1 = nc.dram_tensor("b1", [1, HID], f16, kind="ExternalInput").ap()
    w2 = nc.dram_tensor("w2", [P, KH * OUT], f16, kind="ExternalInput").ap()
    b2 = nc.dram_tensor("b2", [1, OUT], f16, kind="ExternalInput").ap()
    idxA_d = nc.dram_tensor(
        "idxA", [P, T * max(K_A, 1) * 8], i16, kind="ExternalInput").ap()
    idxB_d = nc.dram_tensor(
        "idxB", [P, T * max(K_B, 1) * 8], i16, kind="ExternalInput").ap()
    scm_d = nc.dram_tensor("scm", [P, T * K], f16, kind="ExternalInput").ap()
    dinv_d = nc.dram_tensor("dinv", [P, T], f32, kind="ExternalInput").ap()
    dinv2_d = nc.dram_tensor("dinv2", [P, T], f32, kind="ExternalInput").ap()
    sqdeg_d = nc.dram_tensor("sqdeg", [1, NPC], f16, kind="ExternalInput").ap()
    out = nc.dram_tensor("out", [NPC, OUT], f32, kind="ExternalOutput").ap()

    rg = [list(range(n_cores))]

    with tile.TileContext(nc) as tc:
        with (
            tc.tile_pool(name="dram", bufs=1, space="DRAM") as dpool,
            tc.tile_pool(name="const", bufs=1) as cpool,
            tc.tile_pool(name="pers", bufs=1) as ppool,
            tc.tile_pool(name="work", bufs=3) as wpool,
            tc.tile_pool(name="gath", bufs=8) as gpool,
            tc.tile_pool(name="sgen", bufs=4) as spool,
            tc.tile_pool(name="ps", bufs=2, space="PSUM") as pspool,
            tc.tile_pool(name="psg", bufs=2, space="PSUM") as psgpool,
        ):
            h1s_loc = dpool.tile([NPC, HID], f16)
            h1s_full = dpool.tile([NG, HID], f16, addr_space="Shared")
            h2s_loc = dpool.tile([NPC, OUT], f16)
            h2s_full = dpool.tile([NG, OUT], f16, addr_space="Shared")

            # ---- constants -------------------------------------------------
            w1_sb = cpool.tile([P, KI * HID], f16)
            nc.sync.dma_start(out=w1_sb[:], in_=w1[:])
            w2_sb = cpool.tile([P, KH * OUT], f16)
            nc.sync.dma_start(out=w2_sb[:], in_=w2[:])
            b1_sb = cpool.tile([1, HID], f16)
            nc.sync.dma_start(out=b1_sb[:], in_=b1[:])
            b2_sb = cpool.tile([1, OUT], f16)
            nc.sync.dma_start(out=b2_sb[:], in_=b2[:])
            ident = cpool.tile([P, P], f16)
            make_identity(nc, ident[:])
            idxA_sb = cpool.tile([P, T * max(K_A, 1) * 8], i16)
            nc.sync.dma_start(out=idxA_sb[:], in_=idxA_d[:])
            idxB_sb = cpool.tile([P, T * max(K_B, 1) * 8], i16)
            nc.sync.dma_start(out=idxB_sb[:], in_=idxB_d[:])
            scm_sb = cpool.tile([P, T * K], f16)
            nc.sync.dma_start(out=scm_sb[:], in_=scm_d[:])
            dinv_sb = cpool.tile([P, T], f32)
            nc.sync.dma_start(out=dinv_sb[:], in_=dinv_d[:])
            dinv2_sb = cpool.tile([P, T], f32)
            nc.sync.dma_start(out=dinv2_sb[:], in_=dinv2_d[:])
            sqdeg_sb = cpool.tile([1, NPC], f16)
            nc.sync.dma_start(out=sqdeg_sb[:], in_=sqdeg_d[:])
            # iota_tiled[p, k, d] = d  (fp16-exact for d < 2048); materialized
            # (not broadcast) so the DVE is_equal reads one contiguous stream
            iota_i = cpool.tile([P, P], i16)
            nc.gpsimd.iota(iota_i[:], pattern=[[1, P]], base=0,
                           channel_multiplier=0)
            iota_sb = cpool.tile([P, P], f16)
            nc.vector.tensor_copy(out=iota_sb[:], in_=iota_i[:])
            iota_tiled = cpool.tile([P, K, P], f16)
            nc.vector.tensor_copy(
                out=iota_tiled[:],
                in_=iota_sb[:].rearrange("p (o d) -> p o d", o=1)
                    .broadcast_to([P, K, P]),
            )

            h1s_sb = ppool.tile([P, T * HID], f16)  # local scaled h1 tiles
            h2s_sb = ppool.tile([P, T * OUT], f16)  # local scaled h2 tiles
            xt_all = ppool.tile([P, T * IN_pad], f16)
            nc.sync.dma_start(out=xt_all[:], in_=xlT[:])

            # ---- phase A: h1s = dinv * (x @ W1), local rows ---------------
            for t in range(T):
                xt = xt_all[:, t * IN_pad:(t + 1) * IN_pad]
                ps = pspool.tile([P, HID], f32, tag="ps")
                for ki in range(KI):
                    nc.tensor.matmul(
                        ps[:],
                        lhsT=xt[:, ki * P:(ki + 1) * P],
                        rhs=w1_sb[:, ki * HID:(ki + 1) * HID],
                        start=(ki == 0),
                        stop=(ki == KI - 1),
                    )
                h1t = h1s_sb[:, t * HID:(t + 1) * HID]
                nc.scalar.activation(
                    out=h1t, in_=ps[:], func=Copy,
                    scale=dinv_sb[:, t:t + 1],
                )
                nc.sync.dma_start(
                    out=h1s_loc[t * P:(t + 1) * P, :], in_=h1t
                )

            # ---- phase B: AllGather h1s -----------------------------------
            nc.gpsimd.collective_compute(
                "AllGather",
                mybir.AluOpType.bypass,
                replica_groups=rg,
                ins=[h1s_loc.opt()],
                outs=[h1s_full.opt()],
            )

            def gathers(t, h_full, F):
                """Windowed dma_gathers for dst tile t on rotating queues;
                returns k -> gathered [128, F] slice."""
                gA = gpool.tile([P, max(K_A, 1) * 256], f16, tag="gA",
                                name="gA")
                gB = gpool.tile([P, max(K_B, 1) * 256], f16, tag="gB",
                                name="gB")
                if K_A > 0:
                    nc.gpsimd.dma_gather(
                        out_ap=gA[:, :K_A * F].rearrange(
                            "p (k e) -> p k e", e=F),
                        in_ap=h_full[0:WA, :],
                        idxs_ap=idxA_sb[:, t * K_A * 8:(t + 1) * K_A * 8],
                        num_idxs=K_A * P,
                        num_idxs_reg=K_A * P,
                        elem_size=F,
                        single_packet=False,
                        queue_num=(2 * t) % N_QUEUES,
                    )
                if K_B > 0:
                    nc.gpsimd.dma_gather(
                        out_ap=gB[:, :K_B * F].rearrange(
                            "p (k e) -> p k e", e=F),
                        in_ap=h_full[WB_off:NG, :],
                        idxs_ap=idxB_sb[:, t * K_B * 8:(t + 1) * K_B * 8],
                        num_idxs=K_B * P,
                        num_idxs_reg=K_B * P,
                        elem_size=F,
                        single_packet=False,
                        queue_num=(2 * t + 1) % N_QUEUES,
                    )

                def chunk(k):
                    if k < K_A:
                        return gA[:, k * F:(k + 1) * F]
                    j = k - K_A
                    return gB[:, j * F:(j + 1) * F]

                return chunk

            def gen_s(t):
                """One-hot scatter matrices for tile t: [128, K, 128] fp16."""
                s_sb = spool.tile([P, K, P], f16, tag="s", name="s_sb")
                nc.vector.tensor_tensor(
                    out=s_sb[:],
                    in0=iota_tiled[:],
                    in1=scm_sb[:, t * K:(t + 1) * K]
                        .rearrange("p (k o) -> p k o", o=1)
                        .broadcast_to([P, K, P]),
                    op=mybir.AluOpType.is_equal,
                )
                return s_sb

            # ---- phase C: layer-1 aggregate (transposed) + GEMM2 ----------
            for t in range(T):
                chunk = gathers(t, h1s_full, HID)
                s_sb = gen_s(t)
                aT = wpool.tile([P, KH * P], f16, tag="aT")
                for kh in range(KH):
                    psT = psgpool.tile([P, P], f32, tag=f"psT{kh}")
                    first = True
                    if has_b1:
                        nc.tensor.matmul(
                            psT[:],
                            lhsT=b1_sb[:, kh * P:(kh + 1) * P],
                            rhs=sqdeg_sb[:, t * P:(t + 1) * P],
                            start=True, stop=False,
                        )
                        first = False
                    nc.tensor.matmul(
                        psT[:],
                        lhsT=h1s_sb[:, t * HID + kh * P:
                                    t * HID + (kh + 1) * P],
                        rhs=ident[:],
                        start=first, stop=False,
                    )
                    for k in range(K):
                        nc.tensor.matmul(
                            psT[:],
                            lhsT=chunk(k)[:, kh * P:(kh + 1) * P],
                            rhs=s_sb[:, k, :],
                            start=False, stop=(k == K - 1),
                        )
                    nc.scalar.activation(
                        out=aT[:, kh * P:(kh + 1) * P], in_=psT[:],
                        func=Relu,
                    )
                ps2_full = pspool.tile([P, HID], f32, tag="ps", name="ps2")
                ps2 = ps2_full[:, :OUT]
                for kh in range(KH):
                    nc.tensor.matmul(
                        ps2[:],
                        lhsT=aT[:, kh * P:(kh + 1) * P],
                        rhs=w2_sb[:, kh * OUT:(kh + 1) * OUT],
                        start=(kh == 0),
                        stop=(kh == KH - 1),
                    )
                h2t = h2s_sb[:, t * OUT:(t + 1) * OUT]
                nc.scalar.activation(
                    out=h2t, in_=ps2[:], func=Copy,
                    scale=dinv2_sb[:, t:t + 1],
                )
                nc.sync.dma_start(
                    out=h2s_loc[t * P:(t + 1) * P, :], in_=h2t
                )

            # ---- phase D: AllGather h2s -----------------------------------
            nc.gpsimd.collective_compute(
                "AllGather",
                mybir.AluOpType.bypass,
                replica_groups=rg,
                ins=[h2s_loc.opt()],
                outs=[h2s_full.opt()],
            )

            # ---- phase E: layer-2 aggregate -------------------------------
            for t in range(T):
                chunk = gathers(t, h2s_full, OUT)
                s_sb = gen_s(t)
                ps_full = pspool.tile([P, HID], f32, tag="ps", name="ps")
                ps = ps_full[:, :OUT]
                first = True
                if has_b2:
                    nc.tensor.matmul(
                        ps[:],
                        lhsT=sqdeg_sb[:, t * P:(t + 1) * P],
                        rhs=b2_sb[:],
                        start=True, stop=False,
                    )
                    first = False
                nc.tensor.matmul(
                    ps[:],
                    lhsT=ident[:],
                    rhs=h2s_sb[:, t * OUT:(t + 1) * OUT],
                    start=first, stop=False,
                )
                for k in range(K):
                    nc.tensor.matmul(
                        ps[:],
                        lhsT=s_sb[:, k, :],
                        rhs=chunk(k),
                        start=False, stop=(k == K - 1),
                    )
                ot = wpool.tile([P, OUT], f32, tag="ot")
                nc.scalar.activation(
                    out=ot[:], in_=ps[:], func=Copy,
                    scale=dinv_sb[:, t:t + 1],
                )
                nc.sync.dma_start(out=out[t * P:(t + 1) * P, :], in_=ot[:])

    nc.compile()
    return nc


def _get_program(T, K_A, K_B, KI, HID, OUT, NPC, NG, WA, WB_off,
                 has_b1, has_b2, n_cores=N_CORES):
    key = (T, K_A, K_B, KI, HID, OUT, NPC, NG, WA, WB_off,
           has_b1, has_b2, n_cores)
    if key not in _prog_cache:
        _prog_cache[key] = _build_program(
            T, K_A, K_B, KI, HID, OUT, NPC, NG, WA, WB_off,
            has_b1, has_b2, n_cores
        )
    return _prog_cache[key]


# ------------------------------------------------------------------- driver


def _make_in_maps(x, edge_index, W1, b1, W2, b2):
    W1 = np.asarray(W1, dtype=np.float32)
    W2 = np.asarray(W2, dtype=np.float32)
    b1 = np.asarray(b1, dtype=np.float32).reshape(1, -1)
    b2 = np.asarray(b2, dtype=np.float32).reshape(1, -1)
    arrs, meta = _preprocess(x, edge_index)
    IN_pad = meta["IN_pad"]
    KI = meta["KI"]
    HID = W1.shape[1]
    OUT = W2.shape[1]
    if W1.shape[0] < IN_pad:
        W1 = np.concatenate(
            [W1, np.zeros((IN_pad - W1.shape[0], HID), np.float32)], axis=0
        )
    # device layout: w1 [128, KI*HID] fp16 (chunk ki at cols ki*HID..)
    w1_dev = np.concatenate(
        [W1[ki * P:(ki + 1) * P].astype(np.float16) for ki in range(KI)],
        axis=1,
    )
    KH = HID // P
    w2_dev = np.concatenate(
        [W2[kh * P:(kh + 1) * P].astype(np.float16) for kh in range(KH)],
        axis=1,
    )
    in_maps = [
        {
            "xlT": arrs["xlT"][c],
            "w1": w1_dev,
            "b1": b1.astype(np.float16),
            "w2": w2_dev,
            "b2": b2.astype(np.float16),
            "idxA": arrs["idxA"][c],
            "idxB": arrs["idxB"][c],
            "scm": arrs["scm"][c],
            "dinv": arrs["dinv_col"][c],
            "dinv2": arrs["dinv2_col"][c],
            "sqdeg": arrs["sqdeg_row"][c],
        }
        for c in range(N_CORES)
    ]
    has_b1 = bool(np.any(b1 != 0))
    has_b2 = bool(np.any(b2 != 0))
    return in_maps, meta, HID, OUT, has_b1, has_b2


def run(x, edge_index, W1, b1, W2, b2, trace=False, trace_cores=None):
    from concourse.bass_utils import run_bass_kernel_spmd

    in_maps, meta, HID, OUT, has_b1, has_b2 = _make_in_maps(
        x, edge_index, W1, b1, W2, b2)
    nc = _get_program(
        meta["T"], meta["K_A"], meta["K_B"], meta["KI"], HID, OUT,
        meta["NPC"], meta["NG"], meta["WA"], meta["WB_off"],
        has_b1, has_b2,
    )
    res = run_bass_kernel_spmd(
        nc,
        in_maps,
        core_ids=list(range(N_CORES)),
        trace=trace,
        trace_cores=trace_cores,
    )
    outs = [res.results[c]["out"] for c in range(N_CORES)]
    return _assemble(outs, meta, OUT), res


def kernel(x, edge_index, W1, b1, W2, b2):
    full, _ = run(x, edge_index, W1, b1, W2, b2, trace=False)
    return full


# revision 16
# speedup vs baseline: 2.5018x; 1.0176x over previous
"""Two-layer GCN (PyG GCNConv-style) on 8 Trainium2 NeuronCores.

Strategy: nodes are partitioned across the 8 cores (load-balanced into
128-row destination tiles by in-degree), edges partitioned by destination
node so the segment-sum is local to the destination's core.

Both layers are transform-first (linearity of the GCN aggregation):
  layer 1:  h1 = x @ W1 (local rows)  -> AllGather -> aggregate
  layer 2:  h2 = a @ W2 (local rows)  -> AllGather -> aggregate
The symmetric norm dinv[s]*dinv[d] is split: dinv[src] is folded into the
gathered tables (h1s = dinv*h1, h2s = dinv*h2), dinv[dst] is applied on
the aggregated PSUM via per-partition activation scales (relu commutes
with the positive scale, so layer 1's dst factor rides into the h2 write
as dinv^2).  The per-chunk scatter matrix S is then a pure one-hot
matrix, generated on-device by the Vector engine as
S[e, d] = (iota[d] == dslot[e]) - no S traffic from HBM.  Self loops use
the identity matrix against the SBUF-resident local feature tiles.

Layer-1 aggregation is computed TRANSPOSED (aggT = chunk^T @ S) so the
relu'd result is directly the lhsT of the layer-2 GEMM - no transposes.

dma_gather descriptor generation costs ~8ns/row of GpSimd (Q7) time and
is the fundamental bottleneck (2x100k gathered rows per core).  The
ucode assigns each SWDGE queue to its own Q7 core pair, so gathers issued
round-robin on 4 queues generate descriptors 4x in parallel (measured).

Gathered tables, weights and matmul operands are fp16; PSUM accumulates
fp32.  dma_gather indices are int16, so the 50176-row tables are
addressed through two overlapping 32512-row windows.
"""

import numpy as np

P = 128
N_CORES = 8
WINDOW_CAP = 32512  # dma_gather int16 window (multiple of 128, <= 32767)
N_QUEUES = 4

_prog_cache = {}


# ---------------------------------------------------------------- host side


def _preprocess(x, edge_index):
    """Partition nodes/edges, build per-core device arrays."""
    x = np.asarray(x, dtype=np.float32)
    ei = np.asarray(edge_index)
    N, IN = x.shape

    src = ei[0].astype(np.int64)
    dst = ei[1].astype(np.int64)

    deg = 1 + np.bincount(dst, minlength=N)  # with self loop, >= 1
    dinv = (1.0 / np.sqrt(deg.astype(np.float64))).astype(np.float32)
    sqdeg = np.sqrt(deg.astype(np.float64)).astype(np.float32)

    npc_nodes = -(-N // N_CORES)
    T = -(-npc_nodes // P)  # dst tiles per core
    NPC = T * P  # node slots per core
    n_tiles = N_CORES * T
    NG = n_tiles * P  # global node slots

    # --- pack nodes into tiles, balancing per-tile in-degree (LPT) ----
    import heapq

    degg = deg - 1  # gathered (non-self) in-degree
    tile_of = np.empty(N, dtype=np.int64)
    pos_of = np.empty(N, dtype=np.int64)
    counts = np.zeros(n_tiles, dtype=np.int64)
    loads = np.zeros(n_tiles, dtype=np.int64)
    order = np.argsort(-degg, kind="stable")
    heap = [(0, t) for t in range(n_tiles)]
    heapq.heapify(heap)
    deg_l = degg[order]
    for i in range(N):
        v = order[i]
        while True:
            load, t = heapq.heappop(heap)
            if counts[t] < P:
                break
        tile_of[v] = t
        pos_of[v] = counts[t]
        counts[t] += 1
        load += int(deg_l[i])
        loads[t] = load
        if counts[t] < P:
            heapq.heappush(heap, (load, t))

    # repair pass: move small nodes off overloaded tiles toward the ideal
    # chunk count
    K_ideal = max(1, int(-(-int(degg.sum()) // (n_tiles * P))))
    target = K_ideal * P
    if loads.max() > target:
        by_tile = [[] for _ in range(n_tiles)]
        for i in range(N - 1, -1, -1):  # ascending degree order
            by_tile[tile_of[order[i]]].append(order[i])
        free = [(loads[t], t) for t in range(n_tiles)
                if counts[t] < P and loads[t] < target]
        heapq.heapify(free)
        for t_over in np.flatnonzero(loads > target):
            stack = by_tile[t_over]
            si = 0
            while loads[t_over] > target and si < len(stack) and free:
                v = stack[si]
                si += 1
                d = int(degg[v])
                moved = False
                tried = []
                while free:
                    lo, t2 = heapq.heappop(free)
                    if lo != loads[t2] or counts[t2] >= P:
                        continue  # stale
                    if loads[t2] + d <= target:
                        tile_of[v] = t2
                        pos_of[v] = counts[t2]
                        counts[t2] += 1
                        loads[t2] += d
                        loads[t_over] -= d
                        moved = True
                        if counts[t2] < P and loads[t2] < target:
                            heapq.heappush(free, (loads[t2], t2))
                        break
                    tried.append((lo, t2))
                for it in tried:
                    heapq.heappush(free, it)
                if not moved:
                    break
        # recompute pos_of consistently (holes possible after moves)
        ordv = np.lexsort((np.arange(N), tile_of))
        pos = np.empty(N, dtype=np.int64)
        tt = tile_of[ordv]
        st = np.zeros(n_tiles + 1, dtype=np.int64)
        np.cumsum(np.bincount(tt, minlength=n_tiles), out=st[1:])
        pos[ordv] = np.arange(N) - st[tt]
        pos_of = pos

    K = max(1, int(-(-loads.max() // P)))  # min gather chunks per dst tile

    row_of = tile_of * P + pos_of  # global new row of each node

    # --- per-edge placement (non-self edges) --------------------------
    e_tile = tile_of[dst]
    e_srcrow = row_of[src]

    sort_idx = np.lexsort((e_srcrow, e_tile))
    e_tile = e_tile[sort_idx]
    e_dslot = pos_of[dst][sort_idx].astype(np.int64)
    e_srcrow = e_srcrow[sort_idx]
    nE = len(e_tile)

    # --- window split (dma_gather int16 limit) ------------------------
    WA = min(WINDOW_CAP, NG)  # window A = rows [0, WA)
    WB_off = max(NG - WINDOW_CAP, 0)  # window B = rows [WB_off, NG)
    use_B = WB_off > 0

    tile_n = np.bincount(e_tile, minlength=n_tiles)
    if use_B:
        mustA = e_srcrow < WB_off
        mustB = e_srcrow >= WA
        flex = ~mustA & ~mustB
        cntA = np.bincount(e_tile[mustA], minlength=n_tiles)
        cntB = np.bincount(e_tile[mustB], minlength=n_tiles)
        found = None
        K_tot = K
        while found is None:
            mid = -(-K_tot // 2)
            for d in range(K_tot + 1):
                for K_A in {mid + d, mid - d}:
                    if not 0 <= K_A <= K_tot:
                        continue
                    K_B = K_tot - K_A
                    if (
                        cntA.max() <= K_A * P
                        and cntB.max() <= K_B * P
                        and tile_n.max() <= (K_A + K_B) * P
                    ):
                        found = (K_A, K_B)
                        break
                if found:
                    break
            if not found:
                K_tot += 1
        K_A, K_B = found
        capB = K_B * P
        nA_t = np.minimum(K_A * P, cntA + np.bincount(
            e_tile[flex], minlength=n_tiles))
        nA_t = np.maximum(nA_t, tile_n - capB)
        flexA_quota = nA_t - cntA
        flex_idx = np.flatnonzero(flex)
        ft = e_tile[flex_idx]
        fstart = np.zeros(n_tiles + 1, dtype=np.int64)
        np.cumsum(np.bincount(ft, minlength=n_tiles), out=fstart[1:])
        frank = np.arange(len(ft)) - fstart[ft]
        toA = mustA.copy()
        toA[flex_idx[frank < flexA_quota[ft]]] = True
    else:
        K_A, K_B = K, 0
        toA = np.ones(nE, dtype=bool)
    K_tot = K_A + K_B

    # --- slot assignment within each (tile, window) -------------------
    e_j = np.empty(nE, dtype=np.int64)  # position within its window list
    e_val = np.empty(nE, dtype=np.int64)  # int16 index value
    for is_A in (True, False):
        m = toA if is_A else ~toA
        if not m.any():
            continue
        idxs = np.flatnonzero(m)
        t_sel = e_tile[idxs]
        start = np.zeros(n_tiles + 1, dtype=np.int64)
        np.cumsum(np.bincount(t_sel, minlength=n_tiles), out=start[1:])
        e_j[idxs] = np.arange(len(idxs)) - start[t_sel]
        e_val[idxs] = e_srcrow[idxs] - (0 if is_A else WB_off)

    e_p = e_j % P  # partition (edge slot)
    e_chunk = np.where(toA, e_j // P, K_A + e_j // P)  # chunk within tile

    e_core = e_tile // T
    e_t_in_core = e_tile % T

    # --- idx tables, tile-major: value j at [j%16, t*Kw*8 + j//16] ----
    idxA = np.zeros((N_CORES, 16, T * max(K_A, 1) * 8), dtype=np.int16)
    idxB = np.zeros((N_CORES, 16, T * max(K_B, 1) * 8), dtype=np.int16)
    for arr, sel, Kw in ((idxA, toA, K_A), (idxB, ~toA, K_B)):
        if Kw == 0:
            continue
        m = np.flatnonzero(sel)
        arr[e_core[m], e_j[m] % 16, e_t_in_core[m] * Kw * 8 + e_j[m] // 16] = (
            e_val[m].astype(np.int16)
        )
    idxA = np.tile(idxA, (1, 8, 1))  # [cores, 128, T*K_A*8]
    idxB = np.tile(idxB, (1, 8, 1))

    # --- dslot table: [core, 128, T*K_tot] fp16, padding -1 -----------
    scm = np.full((N_CORES, P, T * K_tot), -1.0, dtype=np.float16)
    scm[e_core, e_p, e_t_in_core * K_tot + e_chunk] = e_dslot.astype(np.float16)

    # --- per-node scale vectors, per core -----------------------------
    n_core = (tile_of // T).astype(np.int64)
    n_t_in_core = tile_of % T
    n_slot = pos_of
    dinv_col = np.zeros((N_CORES, P, T), dtype=np.float32)
    dinv2_col = np.zeros((N_CORES, P, T), dtype=np.float32)
    sqdeg_row = np.zeros((N_CORES, 1, NPC), dtype=np.float16)
    dinv_col[n_core, n_slot, n_t_in_core] = dinv
    dinv2_col[n_core, n_slot, n_t_in_core] = dinv * dinv
    sqdeg_row[n_core, 0, n_t_in_core * P + n_slot] = sqdeg.astype(np.float16)

    # --- per-core transposed node features, fp16, tile-major ----------
    KI = -(-IN // P)
    IN_pad = KI * P
    xf16 = x.astype(np.float16)
    xlT = np.zeros((N_CORES, P, T * IN_pad), dtype=np.float16)
    for ki in range(KI):
        pp = min(P, IN - ki * P)
        cols = n_t_in_core * IN_pad + ki * P + n_slot
        xlT[n_core, :pp, cols] = xf16[:, ki * P:ki * P + pp]

    meta = dict(
        N=N, IN=IN, IN_pad=IN_pad, KI=KI, T=T, K_A=K_A, K_B=K_B, K=K_tot,
        NPC=NPC, NG=NG, WA=WA, WB_off=WB_off,
        node_core=n_core, node_col=n_t_in_core * P + n_slot,
    )
    arrs = dict(
        xlT=xlT, idxA=idxA, idxB=idxB, scm=scm,
        dinv_col=dinv_col, dinv2_col=dinv2_col, sqdeg_row=sqdeg_row,
    )
    return arrs, meta


def _assemble(outs, meta, OUT):
    """Gather per-core outputs back to the original node order."""
    N = meta["N"]
    full = np.empty((N, OUT), dtype=np.float32)
    node_core = meta["node_core"]
    node_col = meta["node_col"]
    for c in range(N_CORES):
        m = node_core == c
        full[m] = outs[c][node_col[m]]
    return full


# -------------------------------------------------------------- device side


def _build_program(T, K_A, K_B, KI, HID, OUT, NPC, NG, WA, WB_off,
                   has_b1, has_b2, n_cores):
    import concourse.bacc as bacc
    import concourse.tile as tile
    from concourse import mybir
    from concourse.masks import make_identity

    f32 = mybir.dt.float32
    f16 = mybir.dt.float16
    i16 = mybir.dt.int16
    K = K_A + K_B
    IN_pad = KI * P
    KH = HID // P  # 128-chunks of hidden dim
    Relu = mybir.ActivationFunctionType.Relu
    Copy = mybir.ActivationFunctionType.Copy

    nc = bacc.Bacc(
        "TRN2", target_bir_lowering=False, debug=False, num_devices=n_cores,
        num_swdge_queues=N_QUEUES,
    )

    xlT = nc.dram_tensor("xlT", [P, T * IN_pad], f16, kind="ExternalInput").ap()
    w1 = nc.dram_tensor("w1", [P, KI * HID], f16, kind="ExternalInput").ap()
    b1 = nc.dram_tensor("b1", [1, HID], f16, kind="ExternalInput").ap()
    w2 = nc.dram_tensor("w2", [P, KH * OUT], f16, kind="ExternalInput").ap()
    b2 = nc.dram_tensor("b2", [1, OUT], f16, kind="ExternalInput").ap()
    idxA_d = nc.dram_tensor(
        "idxA", [P, T * max(K_A, 1) * 8], i16, kind="ExternalInput").ap()
    idxB_d = nc.dram_tensor(
        "idxB", [P, T * max(K_B, 1) * 8], i16, kind="ExternalInput").ap()
    scm_d = nc.dram_tensor("scm", [P, T * K], f16, kind="ExternalInput").ap()
    dinv_d = nc.dram_tensor("dinv", [P, T], f32, kind="ExternalInput").ap()
    dinv2_d = nc.dram_tensor("dinv2", [P, T], f32, kind="ExternalInput").ap()
    sqdeg_d = nc.dram_tensor("sqdeg", [1, NPC], f16, kind="ExternalInput").ap()
    out = nc.dram_tensor("out", [NPC, OUT], f32, kind="ExternalOutput").ap()

    rg = [list(range(n_cores))]

    with tile.TileContext(nc) as tc:
        with (
            tc.tile_pool(name="dram", bufs=1, space="DRAM") as dpool,
            tc.tile_pool(name="const", bufs=1) as cpool,
            tc.tile_pool(name="pers", bufs=1) as ppool,
            tc.tile_pool(name="work", bufs=3) as wpool,
            tc.tile_pool(name="gath", bufs=8) as gpool,
            tc.tile_pool(name="sgen", bufs=4) as spool,
            tc.tile_pool(name="ps", bufs=4, space="PSUM") as pspool,
            tc.tile_pool(name="psg", bufs=2, space="PSUM") as psgpool,
        ):
            h1s_loc = dpool.tile([NPC, HID], f16)
            h1s_full = dpool.tile([NG, HID], f16, addr_space="Shared")
            h2s_loc = dpool.tile([NPC, OUT], f16)
            h2s_full = dpool.tile([NG, OUT], f16, addr_space="Shared")

            # ---- constants -------------------------------------------------
            w1_sb = cpool.tile([P, KI * HID], f16)
            nc.sync.dma_start(out=w1_sb[:], in_=w1[:])
            w2_sb = cpool.tile([P, KH * OUT], f16)
            nc.sync.dma_start(out=w2_sb[:], in_=w2[:])
            b1_sb = cpool.tile([1, HID], f16)
            nc.sync.dma_start(out=b1_sb[:], in_=b1[:])
            b2_sb = cpool.tile([1, OUT], f16)
            nc.sync.dma_start(out=b2_sb[:], in_=b2[:])
            ident = cpool.tile([P, P], f16)
            make_identity(nc, ident[:])
            idxA_sb = cpool.tile([P, T * max(K_A, 1) * 8], i16)
            nc.sync.dma_start(out=idxA_sb[:], in_=idxA_d[:])
            idxB_sb = cpool.tile([P, T * max(K_B, 1) * 8], i16)
            nc.sync.dma_start(out=idxB_sb[:], in_=idxB_d[:])
            scm_sb = cpool.tile([P, T * K], f16)
            nc.sync.dma_start(out=scm_sb[:], in_=scm_d[:])
            dinv_sb = cpool.tile([P, T], f32)
            nc.sync.dma_start(out=dinv_sb[:], in_=dinv_d[:])
            dinv2_sb = cpool.tile([P, T], f32)
            nc.sync.dma_start(out=dinv2_sb[:], in_=dinv2_d[:])
            sqdeg_sb = cpool.tile([1, NPC], f16)
            nc.sync.dma_start(out=sqdeg_sb[:], in_=sqdeg_d[:])
            # iota_tiled[p, k, d] = d  (fp16-exact for d < 2048); materialized
            # (not broadcast) so the DVE is_equal reads one contiguous stream
            iota_i = cpool.tile([P, P], i16)
            nc.gpsimd.iota(iota_i[:], pattern=[[1, P]], base=0,
                           channel_multiplier=0)
            iota_sb = cpool.tile([P, P], f16)
            nc.vector.tensor_copy(out=iota_sb[:], in_=iota_i[:])
            iota_tiled = cpool.tile([P, K, P], f16)
            nc.vector.tensor_copy(
                out=iota_tiled[:],
                in_=iota_sb[:].rearrange("p (o d) -> p o d", o=1)
                    .broadcast_to([P, K, P]),
            )

            h1s_sb = ppool.tile([P, T * HID], f16)  # local scaled h1 tiles
            h2s_sb = ppool.tile([P, T * OUT], f16)  # local scaled h2 tiles
            xt_all = ppool.tile([P, T * IN_pad], f16)
            nc.sync.dma_start(out=xt_all[:], in_=xlT[:])

            # ---- phase A: h1s = dinv * (x @ W1), local rows ---------------
            for t in range(T):
                xt = xt_all[:, t * IN_pad:(t + 1) * IN_pad]
                ps = pspool.tile([P, HID], f32, tag="ps")
                for ki in range(KI):
                    nc.tensor.matmul(
                        ps[:],
                        lhsT=xt[:, ki * P:(ki + 1) * P],
                        rhs=w1_sb[:, ki * HID:(ki + 1) * HID],
                        start=(ki == 0),
                        stop=(ki == KI - 1),
                    )
                h1t = h1s_sb[:, t * HID:(t + 1) * HID]
                nc.scalar.activation(
                    out=h1t, in_=ps[:], func=Copy,
                    scale=dinv_sb[:, t:t + 1],
                )
                nc.sync.dma_start(
                    out=h1s_loc[t * P:(t + 1) * P, :], in_=h1t
                )

            # ---- phase B: AllGather h1s -----------------------------------
            nc.gpsimd.collective_compute(
                "AllGather",
                mybir.AluOpType.bypass,
                replica_groups=rg,
                ins=[h1s_loc.opt()],
                outs=[h1s_full.opt()],
            )

            def gathers(t, h_full, F):
                """Windowed dma_gathers for dst tile t on rotating queues;
                returns k -> gathered [128, F] slice."""
                gA = gpool.tile([P, max(K_A, 1) * 256], f16, tag="gA",
                                name="gA")
                gB = gpool.tile([P, max(K_B, 1) * 256], f16, tag="gB",
                                name="gB")
                if K_A > 0:
                    nc.gpsimd.dma_gather(
                        out_ap=gA[:, :K_A * F].rearrange(
                            "p (k e) -> p k e", e=F),
                        in_ap=h_full[0:WA, :],
                        idxs_ap=idxA_sb[:, t * K_A * 8:(t + 1) * K_A * 8],
                        num_idxs=K_A * P,
                        num_idxs_reg=K_A * P,
                        elem_size=F,
                        single_packet=False,
                        queue_num=(2 * t) % N_QUEUES,
                    )
                if K_B > 0:
                    nc.gpsimd.dma_gather(
                        out_ap=gB[:, :K_B * F].rearrange(
                            "p (k e) -> p k e", e=F),
                        in_ap=h_full[WB_off:NG, :],
                        idxs_ap=idxB_sb[:, t * K_B * 8:(t + 1) * K_B * 8],
                        num_idxs=K_B * P,
                        num_idxs_reg=K_B * P,
                        elem_size=F,
                        single_packet=False,
                        queue_num=(2 * t + 1) % N_QUEUES,
                    )

                def chunk(k):
                    if k < K_A:
                        return gA[:, k * F:(k + 1) * F]
                    j = k - K_A
                    return gB[:, j * F:(j + 1) * F]

                return chunk

            def gen_s(t):
                """One-hot scatter matrices for tile t: [128, K, 128] fp16."""
                s_sb = spool.tile([P, K, P], f16, tag="s", name="s_sb")
                nc.vector.tensor_tensor(
                    out=s_sb[:],
                    in0=iota_tiled[:],
                    in1=scm_sb[:, t * K:(t + 1) * K]
                        .rearrange("p (k o) -> p k o", o=1)
                        .broadcast_to([P, K, P]),
                    op=mybir.AluOpType.is_equal,
                )
                return s_sb

            # ---- phase C: layer-1 aggregate (transposed) + GEMM2 ----------
            for t in range(T):
                chunk = gathers(t, h1s_full, HID)
                s_sb = gen_s(t)
                aT = wpool.tile([P, KH * P], f16, tag="aT")
                for kh in range(KH):
                    psT = psgpool.tile([P, P], f32, tag=f"psT{kh}")
                    first = True
                    if has_b1:
                        nc.tensor.matmul(
                            psT[:],
                            lhsT=b1_sb[:, kh * P:(kh + 1) * P],
                            rhs=sqdeg_sb[:, t * P:(t + 1) * P],
                            start=True, stop=False,
                        )
                        first = False
                    nc.tensor.matmul(
                        psT[:],
                        lhsT=h1s_sb[:, t * HID + kh * P:
                                    t * HID + (kh + 1) * P],
                        rhs=ident[:],
                        start=first, stop=False,
                    )
                    for k in range(K):
                        nc.tensor.matmul(
                            psT[:],
                            lhsT=chunk(k)[:, kh * P:(kh + 1) * P],
                            rhs=s_sb[:, k, :],
                            start=False, stop=(k == K - 1),
                        )
                    nc.scalar.activation(
                        out=aT[:, kh * P:(kh + 1) * P], in_=psT[:],
                        func=Relu,
                    )
                ps2_full = pspool.tile([P, HID], f32, tag="ps", name="ps2")
                ps2 = ps2_full[:, :OUT]
                for kh in range(KH):
                    nc.tensor.matmul(
                        ps2[:],
                        lhsT=aT[:, kh * P:(kh + 1) * P],
                        rhs=w2_sb[:, kh * OUT:(kh + 1) * OUT],
                        start=(kh == 0),
                        stop=(kh == KH - 1),
                    )
                h2t = h2s_sb[:, t * OUT:(t + 1) * OUT]
                nc.scalar.activation(
                    out=h2t, in_=ps2[:], func=Copy,
                    scale=dinv2_sb[:, t:t + 1],
                )
                nc.sync.dma_start(
                    out=h2s_loc[t * P:(t + 1) * P, :], in_=h2t
                )

            # ---- phase D: AllGather h2s -----------------------------------
            nc.gpsimd.collective_compute(
                "AllGather",
                mybir.AluOpType.bypass,
                replica_groups=rg,
                ins=[h2s_loc.opt()],
                outs=[h2s_full.opt()],
            )

            # ---- phase E: layer-2 aggregate -------------------------------
            for t in range(T):
                chunk = gathers(t, h2s_full, OUT)
                s_sb = gen_s(t)
                ps_full = pspool.tile([P, HID], f32, tag="ps", name="ps")
                ps = ps_full[:, :OUT]
                first = True
                if has_b2:
                    nc.tensor.matmul(
                        ps[:],
                        lhsT=sqdeg_sb[:, t * P:(t + 1) * P],
                        rhs=b2_sb[:],
                        start=True, stop=False,
                    )
                    first = False
                nc.tensor.matmul(
                    ps[:],
                    lhsT=ident[:],
                    rhs=h2s_sb[:, t * OUT:(t + 1) * OUT],
                    start=first, stop=False,
                )
                for k in range(K):
                    nc.tensor.matmul(
                        ps[:],
                        lhsT=s_sb[:, k, :],
                        rhs=chunk(k),
                        start=False, stop=(k == K - 1),
                    )
                ot = wpool.tile([P, OUT], f32, tag="ot")
                nc.scalar.activation(
                    out=ot[:], in_=ps[:], func=Copy,
                    scale=dinv_sb[:, t:t + 1],
                )
                nc.sync.dma_start(out=out[t * P:(t + 1) * P, :], in_=ot[:])

    nc.compile()
    return nc


def _get_program(T, K_A, K_B, KI, HID, OUT, NPC, NG, WA, WB_off,
                 has_b1, has_b2, n_cores=N_CORES):
    key = (T, K_A, K_B, KI, HID, OUT, NPC, NG, WA, WB_off,
           has_b1, has_b2, n_cores)
    if key not in _prog_cache:
        _prog_cache[key] = _build_program(
            T, K_A, K_B, KI, HID, OUT, NPC, NG, WA, WB_off,
            has_b1, has_b2, n_cores
        )
    return _prog_cache[key]


# ------------------------------------------------------------------- driver


def _make_in_maps(x, edge_index, W1, b1, W2, b2):
    W1 = np.asarray(W1, dtype=np.float32)
    W2 = np.asarray(W2, dtype=np.float32)
    b1 = np.asarray(b1, dtype=np.float32).reshape(1, -1)
    b2 = np.asarray(b2, dtype=np.float32).reshape(1, -1)
    arrs, meta = _preprocess(x, edge_index)
    IN_pad = meta["IN_pad"]
    KI = meta["KI"]
    HID = W1.shape[1]
    OUT = W2.shape[1]
    if W1.shape[0] < IN_pad:
        W1 = np.concatenate(
            [W1, np.zeros((IN_pad - W1.shape[0], HID), np.float32)], axis=0
        )
    # device layout: w1 [128, KI*HID] fp16 (chunk ki at cols ki*HID..)
    w1_dev = np.concatenate(
        [W1[ki * P:(ki + 1) * P].astype(np.float16) for ki in range(KI)],
        axis=1,
    )
    KH = HID // P
    w2_dev = np.concatenate(
        [W2[kh * P:(kh + 1) * P].astype(np.float16) for kh in range(KH)],
        axis=1,
    )
    in_maps = [
        {
            "xlT": arrs["xlT"][c],
            "w1": w1_dev,
            "b1": b1.astype(np.float16),
            "w2": w2_dev,
            "b2": b2.astype(np.float16),
            "idxA": arrs["idxA"][c],
            "idxB": arrs["idxB"][c],
            "scm": arrs["scm"][c],
            "dinv": arrs["dinv_col"][c],
            "dinv2": arrs["dinv2_col"][c],
            "sqdeg": arrs["sqdeg_row"][c],
        }
        for c in range(N_CORES)
    ]
    has_b1 = bool(np.any(b1 != 0))
    has_b2 = bool(np.any(b2 != 0))
    return in_maps, meta, HID, OUT, has_b1, has_b2


def run(x, edge_index, W1, b1, W2, b2, trace=False, trace_cores=None):
    from concourse.bass_utils import run_bass_kernel_spmd

    in_maps, meta, HID, OUT, has_b1, has_b2 = _make_in_maps(
        x, edge_index, W1, b1, W2, b2)
    nc = _get_program(
        meta["T"], meta["K_A"], meta["K_B"], meta["KI"], HID, OUT,
        meta["NPC"], meta["NG"], meta["WA"], meta["WB_off"],
        has_b1, has_b2,
    )
    res = run_bass_kernel_spmd(
        nc,
        in_maps,
        core_ids=list(range(N_CORES)),
        trace=trace,
        trace_cores=trace_cores,
    )
    outs = [res.results[c]["out"] for c in range(N_CORES)]
    return _assemble(outs, meta, OUT), res


def kernel(x, edge_index, W1, b1, W2, b2):
    full, _ = run(x, edge_index, W1, b1, W2, b2, trace=False)
    return full


# revision 18
# speedup vs baseline: 2.5226x; 1.0083x over previous
"""Two-layer GCN (PyG GCNConv-style) on 8 Trainium2 NeuronCores.

Strategy: nodes are partitioned across the 8 cores (load-balanced into
128-row destination tiles by in-degree), edges partitioned by destination
node so the segment-sum is local to the destination's core.

Both layers are transform-first (linearity of the GCN aggregation):
  layer 1:  h1 = x @ W1 (local rows)  -> AllGather -> aggregate
  layer 2:  h2 = a @ W2 (local rows)  -> AllGather -> aggregate
The symmetric norm dinv[s]*dinv[d] is split: dinv[src] is folded into the
gathered tables (h1s = dinv*h1, h2s = dinv*h2), dinv[dst] is applied on
the aggregated PSUM via per-partition activation scales (relu commutes
with the positive scale, so layer 1's dst factor rides into the h2 write
as dinv^2).  The per-chunk scatter matrix S is then a pure one-hot
matrix, generated on-device by the Vector engine as
S[e, d] = (iota[d] == dslot[e]) - no S traffic from HBM.  Self loops use
the identity matrix against the SBUF-resident local feature tiles.

Layer-1 aggregation is computed TRANSPOSED (aggT = chunk^T @ S) so the
relu'd result is directly the lhsT of the layer-2 GEMM - no transposes.

dma_gather descriptor generation costs ~8ns/row of GpSimd (Q7) time and
is the fundamental bottleneck (2x100k gathered rows per core).  The
ucode assigns each SWDGE queue to its own Q7 core pair, so gathers issued
round-robin on 4 queues generate descriptors 4x in parallel (measured).

Gathered tables, weights and matmul operands are fp16; PSUM accumulates
fp32.  dma_gather indices are int16, so the 50176-row tables are
addressed through two overlapping 32512-row windows.
"""

import numpy as np

P = 128
N_CORES = 8
WINDOW_CAP = 32512  # dma_gather int16 window (multiple of 128, <= 32767)
N_QUEUES = 4

_prog_cache = {}


# ---------------------------------------------------------------- host side


def _preprocess(x, edge_index):
    """Partition nodes/edges, build per-core device arrays."""
    x = np.asarray(x, dtype=np.float32)
    ei = np.asarray(edge_index)
    N, IN = x.shape

    src = ei[0].astype(np.int64)
    dst = ei[1].astype(np.int64)

    deg = 1 + np.bincount(dst, minlength=N)  # with self loop, >= 1
    dinv = (1.0 / np.sqrt(deg.astype(np.float64))).astype(np.float32)
    sqdeg = np.sqrt(deg.astype(np.float64)).astype(np.float32)

    npc_nodes = -(-N // N_CORES)
    T = -(-npc_nodes // P)  # dst tiles per core
    NPC = T * P  # node slots per core
    n_tiles = N_CORES * T
    NG = n_tiles * P  # global node slots

    # --- pack nodes into tiles, balancing per-tile in-degree (LPT) ----
    import heapq

    degg = deg - 1  # gathered (non-self) in-degree
    tile_of = np.empty(N, dtype=np.int64)
    pos_of = np.empty(N, dtype=np.int64)
    counts = np.zeros(n_tiles, dtype=np.int64)
    loads = np.zeros(n_tiles, dtype=np.int64)
    order = np.argsort(-degg, kind="stable")
    heap = [(0, t) for t in range(n_tiles)]
    heapq.heapify(heap)
    deg_l = degg[order]
    for i in range(N):
        v = order[i]
        while True:
            load, t = heapq.heappop(heap)
            if counts[t] < P:
                break
        tile_of[v] = t
        pos_of[v] = counts[t]
        counts[t] += 1
        load += int(deg_l[i])
        loads[t] = load
        if counts[t] < P:
            heapq.heappush(heap, (load, t))

    # repair pass: move small nodes off overloaded tiles toward the ideal
    # chunk count
    K_ideal = max(1, int(-(-int(degg.sum()) // (n_tiles * P))))
    target = K_ideal * P
    if loads.max() > target:
        by_tile = [[] for _ in range(n_tiles)]
        for i in range(N - 1, -1, -1):  # ascending degree order
            by_tile[tile_of[order[i]]].append(order[i])
        free = [(loads[t], t) for t in range(n_tiles)
                if counts[t] < P and loads[t] < target]
        heapq.heapify(free)
        for t_over in np.flatnonzero(loads > target):
            stack = by_tile[t_over]
            si = 0
            while loads[t_over] > target and si < len(stack) and free:
                v = stack[si]
                si += 1
                d = int(degg[v])
                moved = False
                tried = []
                while free:
                    lo, t2 = heapq.heappop(free)
                    if lo != loads[t2] or counts[t2] >= P:
                        continue  # stale
                    if loads[t2] + d <= target:
                        tile_of[v] = t2
                        pos_of[v] = counts[t2]
                        counts[t2] += 1
                        loads[t2] += d
                        loads[t_over] -= d
                        moved = True
                        if counts[t2] < P and loads[t2] < target:
                            heapq.heappush(free, (loads[t2], t2))
                        break
                    tried.append((lo, t2))
                for it in tried:
                    heapq.heappush(free, it)
                if not moved:
                    break
        # recompute pos_of consistently (holes possible after moves)
        ordv = np.lexsort((np.arange(N), tile_of))
        pos = np.empty(N, dtype=np.int64)
        tt = tile_of[ordv]
        st = np.zeros(n_tiles + 1, dtype=np.int64)
        np.cumsum(np.bincount(tt, minlength=n_tiles), out=st[1:])
        pos[ordv] = np.arange(N) - st[tt]
        pos_of = pos

    K = max(1, int(-(-loads.max() // P)))  # min gather chunks per dst tile

    row_of = tile_of * P + pos_of  # global new row of each node

    # --- per-edge placement (non-self edges) --------------------------
    e_tile = tile_of[dst]
    e_srcrow = row_of[src]

    sort_idx = np.lexsort((e_srcrow, e_tile))
    e_tile = e_tile[sort_idx]
    e_dslot = pos_of[dst][sort_idx].astype(np.int64)
    e_srcrow = e_srcrow[sort_idx]
    nE = len(e_tile)

    # --- window split (dma_gather int16 limit) ------------------------
    WA = min(WINDOW_CAP, NG)  # window A = rows [0, WA)
    WB_off = max(NG - WINDOW_CAP, 0)  # window B = rows [WB_off, NG)
    use_B = WB_off > 0

    tile_n = np.bincount(e_tile, minlength=n_tiles)
    if use_B:
        mustA = e_srcrow < WB_off
        mustB = e_srcrow >= WA
        flex = ~mustA & ~mustB
        cntA = np.bincount(e_tile[mustA], minlength=n_tiles)
        cntB = np.bincount(e_tile[mustB], minlength=n_tiles)
        found = None
        K_tot = K
        while found is None:
            mid = -(-K_tot // 2)
            for d in range(K_tot + 1):
                for K_A in {mid + d, mid - d}:
                    if not 0 <= K_A <= K_tot:
                        continue
                    K_B = K_tot - K_A
                    if (
                        cntA.max() <= K_A * P
                        and cntB.max() <= K_B * P
                        and tile_n.max() <= (K_A + K_B) * P
                    ):
                        found = (K_A, K_B)
                        break
                if found:
                    break
            if not found:
                K_tot += 1
        K_A, K_B = found
        capB = K_B * P
        nA_t = np.minimum(K_A * P, cntA + np.bincount(
            e_tile[flex], minlength=n_tiles))
        nA_t = np.maximum(nA_t, tile_n - capB)
        flexA_quota = nA_t - cntA
        flex_idx = np.flatnonzero(flex)
        ft = e_tile[flex_idx]
        fstart = np.zeros(n_tiles + 1, dtype=np.int64)
        np.cumsum(np.bincount(ft, minlength=n_tiles), out=fstart[1:])
        frank = np.arange(len(ft)) - fstart[ft]
        toA = mustA.copy()
        toA[flex_idx[frank < flexA_quota[ft]]] = True
    else:
        K_A, K_B = K, 0
        toA = np.ones(nE, dtype=bool)
    K_tot = K_A + K_B

    # --- slot assignment within each (tile, window) -------------------
    e_j = np.empty(nE, dtype=np.int64)  # position within its window list
    e_val = np.empty(nE, dtype=np.int64)  # int16 index value
    for is_A in (True, False):
        m = toA if is_A else ~toA
        if not m.any():
            continue
        idxs = np.flatnonzero(m)
        t_sel = e_tile[idxs]
        start = np.zeros(n_tiles + 1, dtype=np.int64)
        np.cumsum(np.bincount(t_sel, minlength=n_tiles), out=start[1:])
        e_j[idxs] = np.arange(len(idxs)) - start[t_sel]
        e_val[idxs] = e_srcrow[idxs] - (0 if is_A else WB_off)

    e_p = e_j % P  # partition (edge slot)
    e_chunk = np.where(toA, e_j // P, K_A + e_j // P)  # chunk within tile

    e_core = e_tile // T
    e_t_in_core = e_tile % T

    # --- idx tables, tile-major: value j at [j%16, t*Kw*8 + j//16] ----
    idxA = np.zeros((N_CORES, 16, T * max(K_A, 1) * 8), dtype=np.int16)
    idxB = np.zeros((N_CORES, 16, T * max(K_B, 1) * 8), dtype=np.int16)
    for arr, sel, Kw in ((idxA, toA, K_A), (idxB, ~toA, K_B)):
        if Kw == 0:
            continue
        m = np.flatnonzero(sel)
        arr[e_core[m], e_j[m] % 16, e_t_in_core[m] * Kw * 8 + e_j[m] // 16] = (
            e_val[m].astype(np.int16)
        )
    idxA = np.tile(idxA, (1, 8, 1))  # [cores, 128, T*K_A*8]
    idxB = np.tile(idxB, (1, 8, 1))

    # --- dslot table: [core, 128, T*K_tot] fp16, padding -1 -----------
    scm = np.full((N_CORES, P, T * K_tot), -1.0, dtype=np.float16)
    scm[e_core, e_p, e_t_in_core * K_tot + e_chunk] = e_dslot.astype(np.float16)

    # --- per-node scale vectors, per core -----------------------------
    n_core = (tile_of // T).astype(np.int64)
    n_t_in_core = tile_of % T
    n_slot = pos_of
    dinv_col = np.zeros((N_CORES, P, T), dtype=np.float32)
    dinv2_col = np.zeros((N_CORES, P, T), dtype=np.float32)
    sqdeg_row = np.zeros((N_CORES, 1, NPC), dtype=np.float16)
    dinv_col[n_core, n_slot, n_t_in_core] = dinv
    dinv2_col[n_core, n_slot, n_t_in_core] = dinv * dinv
    sqdeg_row[n_core, 0, n_t_in_core * P + n_slot] = sqdeg.astype(np.float16)

    # --- per-core transposed node features, fp16, tile-major ----------
    KI = -(-IN // P)
    IN_pad = KI * P
    xf16 = x.astype(np.float16)
    xlT = np.zeros((N_CORES, P, T * IN_pad), dtype=np.float16)
    for ki in range(KI):
        pp = min(P, IN - ki * P)
        cols = n_t_in_core * IN_pad + ki * P + n_slot
        xlT[n_core, :pp, cols] = xf16[:, ki * P:ki * P + pp]

    meta = dict(
        N=N, IN=IN, IN_pad=IN_pad, KI=KI, T=T, K_A=K_A, K_B=K_B, K=K_tot,
        NPC=NPC, NG=NG, WA=WA, WB_off=WB_off,
        node_core=n_core, node_col=n_t_in_core * P + n_slot,
    )
    arrs = dict(
        xlT=xlT, idxA=idxA, idxB=idxB, scm=scm,
        dinv_col=dinv_col, dinv2_col=dinv2_col, sqdeg_row=sqdeg_row,
    )
    return arrs, meta


def _assemble(outs, meta, OUT):
    """Gather per-core outputs back to the original node order."""
    N = meta["N"]
    full = np.empty((N, OUT), dtype=np.float32)
    node_core = meta["node_core"]
    node_col = meta["node_col"]
    for c in range(N_CORES):
        m = node_core == c
        full[m] = outs[c][node_col[m]]
    return full


# -------------------------------------------------------------- device side


def _build_program(T, K_A, K_B, KI, HID, OUT, NPC, NG, WA, WB_off,
                   has_b1, has_b2, n_cores):
    import concourse.bacc as bacc
    import concourse.tile as tile
    from concourse import mybir
    from concourse.masks import make_identity

    f32 = mybir.dt.float32
    f16 = mybir.dt.float16
    i16 = mybir.dt.int16
    K = K_A + K_B
    IN_pad = KI * P
    KH = HID // P  # 128-chunks of hidden dim
    Relu = mybir.ActivationFunctionType.Relu
    Copy = mybir.ActivationFunctionType.Copy

    nc = bacc.Bacc(
        "TRN2", target_bir_lowering=False, debug=False, num_devices=n_cores,
        num_swdge_queues=N_QUEUES,
    )

    xlT = nc.dram_tensor("xlT", [P, T * IN_pad], f16, kind="ExternalInput").ap()
    w1 = nc.dram_tensor("w1", [P, KI * HID], f16, kind="ExternalInput").ap()
    b1 = nc.dram_tensor("b1", [1, HID], f16, kind="ExternalInput").ap()
    w2 = nc.dram_tensor("w2", [P, KH * OUT], f16, kind="ExternalInput").ap()
    b2 = nc.dram_tensor("b2", [1, OUT], f16, kind="ExternalInput").ap()
    idxA_d = nc.dram_tensor(
        "idxA", [P, T * max(K_A, 1) * 8], i16, kind="ExternalInput").ap()
    idxB_d = nc.dram_tensor(
        "idxB", [P, T * max(K_B, 1) * 8], i16, kind="ExternalInput").ap()
    scm_d = nc.dram_tensor("scm", [P, T * K], f16, kind="ExternalInput").ap()
    dinv_d = nc.dram_tensor("dinv", [P, T], f32, kind="ExternalInput").ap()
    dinv2_d = nc.dram_tensor("dinv2", [P, T], f32, kind="ExternalInput").ap()
    sqdeg_d = nc.dram_tensor("sqdeg", [1, NPC], f16, kind="ExternalInput").ap()
    out = nc.dram_tensor("out", [NPC, OUT], f32, kind="ExternalOutput").ap()

    rg = [list(range(n_cores))]

    with tile.TileContext(nc) as tc:
        with (
            tc.tile_pool(name="dram", bufs=1, space="DRAM") as dpool,
            tc.tile_pool(name="const", bufs=1) as cpool,
            tc.tile_pool(name="pers", bufs=1) as ppool,
            tc.tile_pool(name="work", bufs=3) as wpool,
            tc.tile_pool(name="gath", bufs=8) as gpool,
            tc.tile_pool(name="sgen", bufs=4) as spool,
            tc.tile_pool(name="ps", bufs=4, space="PSUM") as pspool,
            tc.tile_pool(name="psg", bufs=2, space="PSUM") as psgpool,
        ):
            h1s_loc = dpool.tile([NPC, HID], f16)
            h1s_full = dpool.tile([NG, HID], f16, addr_space="Shared")
            h2s_loc = dpool.tile([NPC, OUT], f16)
            h2s_full = dpool.tile([NG, OUT], f16, addr_space="Shared")

            # ---- constants -------------------------------------------------
            w1_sb = cpool.tile([P, KI * HID], f16)
            nc.sync.dma_start(out=w1_sb[:], in_=w1[:])
            w2_sb = cpool.tile([P, KH * OUT], f16)
            nc.sync.dma_start(out=w2_sb[:], in_=w2[:])
            b1_sb = cpool.tile([1, HID], f16)
            nc.sync.dma_start(out=b1_sb[:], in_=b1[:])
            b2_sb = cpool.tile([1, OUT], f16)
            nc.sync.dma_start(out=b2_sb[:], in_=b2[:])
            ident = cpool.tile([P, P], f16)
            make_identity(nc, ident[:])
            idxA_sb = cpool.tile([P, T * max(K_A, 1) * 8], i16)
            nc.sync.dma_start(out=idxA_sb[:], in_=idxA_d[:])
            idxB_sb = cpool.tile([P, T * max(K_B, 1) * 8], i16)
            nc.sync.dma_start(out=idxB_sb[:], in_=idxB_d[:])
            scm_sb = cpool.tile([P, T * K], f16)
            nc.sync.dma_start(out=scm_sb[:], in_=scm_d[:])
            dinv_sb = cpool.tile([P, T], f32)
            nc.sync.dma_start(out=dinv_sb[:], in_=dinv_d[:])
            dinv2_sb = cpool.tile([P, T], f32)
            nc.sync.dma_start(out=dinv2_sb[:], in_=dinv2_d[:])
            sqdeg_sb = cpool.tile([1, NPC], f16)
            nc.sync.dma_start(out=sqdeg_sb[:], in_=sqdeg_d[:])
            # iota_tiled[p, k, d] = d  (fp16-exact for d < 2048); materialized
            # (not broadcast) so the DVE is_equal reads one contiguous stream
            iota_i = cpool.tile([P, P], i16)
            nc.gpsimd.iota(iota_i[:], pattern=[[1, P]], base=0,
                           channel_multiplier=0)
            iota_sb = cpool.tile([P, P], f16)
            nc.vector.tensor_copy(out=iota_sb[:], in_=iota_i[:])
            iota_tiled = cpool.tile([P, K, P], f16)
            nc.vector.tensor_copy(
                out=iota_tiled[:],
                in_=iota_sb[:].rearrange("p (o d) -> p o d", o=1)
                    .broadcast_to([P, K, P]),
            )

            h1s_sb = ppool.tile([P, T * HID], f16)  # local scaled h1 tiles
            h2s_sb = ppool.tile([P, T * OUT], f16)  # local scaled h2 tiles
            xt_all = ppool.tile([P, T * IN_pad], f16)
            nc.sync.dma_start(out=xt_all[:], in_=xlT[:])

            # ---- phase A: h1s = dinv * (x @ W1), local rows ---------------
            for t in range(T):
                xt = xt_all[:, t * IN_pad:(t + 1) * IN_pad]
                ps = pspool.tile([P, HID], f32, tag="ps")
                for ki in range(KI):
                    nc.tensor.matmul(
                        ps[:],
                        lhsT=xt[:, ki * P:(ki + 1) * P],
                        rhs=w1_sb[:, ki * HID:(ki + 1) * HID],
                        start=(ki == 0),
                        stop=(ki == KI - 1),
                    )
                h1t = h1s_sb[:, t * HID:(t + 1) * HID]
                nc.scalar.activation(
                    out=h1t, in_=ps[:], func=Copy,
                    scale=dinv_sb[:, t:t + 1],
                )
                nc.sync.dma_start(
                    out=h1s_loc[t * P:(t + 1) * P, :], in_=h1t
                )

            # ---- phase B: AllGather h1s -----------------------------------
            nc.gpsimd.collective_compute(
                "AllGather",
                mybir.AluOpType.bypass,
                replica_groups=rg,
                ins=[h1s_loc.opt()],
                outs=[h1s_full.opt()],
            )

            def gathers(t, h_full, F):
                """Windowed dma_gathers for dst tile t on rotating queues;
                returns k -> gathered [128, F] slice."""
                gA = gpool.tile([P, max(K_A, 1) * 256], f16, tag="gA",
                                name="gA")
                gB = gpool.tile([P, max(K_B, 1) * 256], f16, tag="gB",
                                name="gB")
                if K_A > 0:
                    nc.gpsimd.dma_gather(
                        out_ap=gA[:, :K_A * F].rearrange(
                            "p (k e) -> p k e", e=F),
                        in_ap=h_full[0:WA, :],
                        idxs_ap=idxA_sb[:, t * K_A * 8:(t + 1) * K_A * 8],
                        num_idxs=K_A * P,
                        num_idxs_reg=K_A * P,
                        elem_size=F,
                        single_packet=False,
                        queue_num=(2 * t) % N_QUEUES,
                    )
                if K_B > 0:
                    nc.gpsimd.dma_gather(
                        out_ap=gB[:, :K_B * F].rearrange(
                            "p (k e) -> p k e", e=F),
                        in_ap=h_full[WB_off:NG, :],
                        idxs_ap=idxB_sb[:, t * K_B * 8:(t + 1) * K_B * 8],
                        num_idxs=K_B * P,
                        num_idxs_reg=K_B * P,
                        elem_size=F,
                        single_packet=False,
                        queue_num=(2 * t + 1) % N_QUEUES,
                    )

                def chunk(k):
                    if k < K_A:
                        return gA[:, k * F:(k + 1) * F]
                    j = k - K_A
                    return gB[:, j * F:(j + 1) * F]

                return chunk

            def gen_s(t):
                """One-hot scatter matrices for tile t: [128, K, 128] fp16."""
                s_sb = spool.tile([P, K, P], f16, tag="s", name="s_sb")
                nc.vector.tensor_tensor(
                    out=s_sb[:],
                    in0=iota_tiled[:],
                    in1=scm_sb[:, t * K:(t + 1) * K]
                        .rearrange("p (k o) -> p k o", o=1)
                        .broadcast_to([P, K, P]),
                    op=mybir.AluOpType.is_equal,
                )
                return s_sb

            # ---- phase C: layer-1 aggregate (transposed) + GEMM2 ----------
            for t in range(T):
                chunk = gathers(t, h1s_full, HID)
                s_sb = gen_s(t)
                aT = wpool.tile([P, KH * P], f16, tag="aT")
                for kh in range(KH):
                    psT = psgpool.tile([P, P], f32, tag=f"psT{kh}")
                    first = True
                    if has_b1:
                        nc.tensor.matmul(
                            psT[:],
                            lhsT=b1_sb[:, kh * P:(kh + 1) * P],
                            rhs=sqdeg_sb[:, t * P:(t + 1) * P],
                            start=True, stop=False,
                        )
                        first = False
                    nc.tensor.matmul(
                        psT[:],
                        lhsT=h1s_sb[:, t * HID + kh * P:
                                    t * HID + (kh + 1) * P],
                        rhs=ident[:],
                        start=first, stop=False,
                    )
                    for k in range(K):
                        nc.tensor.matmul(
                            psT[:],
                            lhsT=chunk(k)[:, kh * P:(kh + 1) * P],
                            rhs=s_sb[:, k, :],
                            start=False, stop=(k == K - 1),
                        )
                    nc.scalar.activation(
                        out=aT[:, kh * P:(kh + 1) * P], in_=psT[:],
                        func=Relu,
                    )
                ps2_full = pspool.tile([P, HID], f32, tag="ps", name="ps2")
                ps2 = ps2_full[:, :OUT]
                for kh in range(KH):
                    nc.tensor.matmul(
                        ps2[:],
                        lhsT=aT[:, kh * P:(kh + 1) * P],
                        rhs=w2_sb[:, kh * OUT:(kh + 1) * OUT],
                        start=(kh == 0),
                        stop=(kh == KH - 1),
                    )
                h2t = h2s_sb[:, t * OUT:(t + 1) * OUT]
                nc.scalar.activation(
                    out=h2t, in_=ps2[:], func=Copy,
                    scale=dinv2_sb[:, t:t + 1],
                )
                nc.sync.dma_start(
                    out=h2s_loc[t * P:(t + 1) * P, :], in_=h2t
                )

            # ---- phase D: AllGather h2s -----------------------------------
            nc.gpsimd.collective_compute(
                "AllGather",
                mybir.AluOpType.bypass,
                replica_groups=rg,
                ins=[h2s_loc.opt()],
                outs=[h2s_full.opt()],
            )

            # ---- phase E: layer-2 aggregate -------------------------------
            for t in range(T):
                chunk = gathers(t, h2s_full, OUT)
                s_sb = gen_s(t)
                ps_full = pspool.tile([P, HID], f32, tag="ps", name="ps")
                ps = ps_full[:, :OUT]
                first = True
                if has_b2:
                    nc.tensor.matmul(
                        ps[:],
                        lhsT=sqdeg_sb[:, t * P:(t + 1) * P],
                        rhs=b2_sb[:],
                        start=True, stop=False,
                    )
                    first = False
                nc.tensor.matmul(
                    ps[:],
                    lhsT=ident[:],
                    rhs=h2s_sb[:, t * OUT:(t + 1) * OUT],
                    start=first, stop=False,
                )
                for k in range(K):
                    nc.tensor.matmul(
                        ps[:],
                        lhsT=s_sb[:, k, :],
                        rhs=chunk(k),
                        start=False, stop=(k == K - 1),
                    )
                ot = wpool.tile([P, OUT], f32, tag="ot")
                nc.scalar.activation(
                    out=ot[:], in_=ps[:], func=Copy,
                    scale=dinv_sb[:, t:t + 1],
                )
                nc.sync.dma_start(out=out[t * P:(t + 1) * P, :], in_=ot[:])

    nc.compile()
    return nc


def _get_program(T, K_A, K_B, KI, HID, OUT, NPC, NG, WA, WB_off,
                 has_b1, has_b2, n_cores=N_CORES):
    key = (T, K_A, K_B, KI, HID, OUT, NPC, NG, WA, WB_off,
           has_b1, has_b2, n_cores)
    if key not in _prog_cache:
        _prog_cache[key] = _build_program(
            T, K_A, K_B, KI, HID, OUT, NPC, NG, WA, WB_off,
            has_b1, has_b2, n_cores
        )
    return _prog_cache[key]


# ------------------------------------------------------------------- driver


def _make_in_maps(x, edge_index, W1, b1, W2, b2):
    W1 = np.asarray(W1, dtype=np.float32)
    W2 = np.asarray(W2, dtype=np.float32)
    b1 = np.asarray(b1, dtype=np.float32).reshape(1, -1)
    b2 = np.asarray(b2, dtype=np.float32).reshape(1, -1)
    arrs, meta = _preprocess(x, edge_index)
    IN_pad = meta["IN_pad"]
    KI = meta["KI"]
    HID = W1.shape[1]
    OUT = W2.shape[1]
    if W1.shape[0] < IN_pad:
        W1 = np.concatenate(
            [W1, np.zeros((IN_pad - W1.shape[0], HID), np.float32)], axis=0
        )
    # device layout: w1 [128, KI*HID] fp16 (chunk ki at cols ki*HID..)
    w1_dev = np.concatenate(
        [W1[ki * P:(ki + 1) * P].astype(np.float16) for ki in range(KI)],
        axis=1,
    )
    KH = HID // P
    w2_dev = np.concatenate(
        [W2[kh * P:(kh + 1) * P].astype(np.float16) for kh in range(KH)],
        axis=1,
    )
    in_maps = [
        {
            "xlT": arrs["xlT"][c],
            "w1": w1_dev,
            "b1": b1.astype(np.float16),
            "w2": w2_dev,
            "b2": b2.astype(np.float16),
            "idxA": arrs["idxA"][c],
            "idxB": arrs["idxB"][c],
            "scm": arrs["scm"][c],
            "dinv": arrs["dinv_col"][c],
            "dinv2": arrs["dinv2_col"][c],
            "sqdeg": arrs["sqdeg_row"][c],
        }
        for c in range(N_CORES)
    ]
    has_b1 = bool(np.any(b1 != 0))
    has_b2 = bool(np.any(b2 != 0))
    return in_maps, meta, HID, OUT, has_b1, has_b2


def run(x, edge_index, W1, b1, W2, b2, trace=False, trace_cores=None):
    from concourse.bass_utils import run_bass_kernel_spmd

    in_maps, meta, HID, OUT, has_b1, has_b2 = _make_in_maps(
        x, edge_index, W1, b1, W2, b2)
    nc = _get_program(
        meta["T"], meta["K_A"], meta["K_B"], meta["KI"], HID, OUT,
        meta["NPC"], meta["NG"], meta["WA"], meta["WB_off"],
        has_b1, has_b2,
    )
    res = run_bass_kernel_spmd(
        nc,
        in_maps,
        core_ids=list(range(N_CORES)),
        trace=trace,
        trace_cores=trace_cores,
    )
    outs = [res.results[c]["out"] for c in range(N_CORES)]
    return _assemble(outs, meta, OUT), res


def kernel(x, edge_index, W1, b1, W2, b2):
    full, _ = run(x, edge_index, W1, b1, W2, b2, trace=False)
    return full
